# revision 7
# baseline (speedup 1.0000x reference)
"""Multi-head attention block on 8 TRN2 NeuronCores.

Sharding: data-parallel over (batch b, query-half h) -> core i = 2*b + h.
Each core computes Q for its 1024 query tokens and K/V for the full 2048-token
sequence of its batch locally (duplicated across the pair), so no collectives
are needed.

Per-core layout choices:
- X is passed host-transposed as XT = X[b].T [D, S] f32, rolled so the core's
  own query tokens occupy columns 0:1023 (attention is permutation-invariant
  over key positions, so the roll is harmless for K/V).
- All matmuls run in bf16 (1 cyc/row on PE vs 4 for fp32), accumulating f32
  in PSUM.
- Scores are computed transposed (scoresT[k, q] = K @ Q^T) so softmax exp is a
  single ACT pass per key-chunk and the denominator falls out of the PV matmul
  via a ones column appended to V ([v_h | 1] lhsT -> psum row 64 = sum_k exp).
"""

import numpy as np

import concourse.bass as bass
import concourse.tile as tile
from concourse import bacc, mybir
from concourse.bass_utils import run_bass_kernel_spmd

F32 = mybir.dt.float32
BF16 = mybir.dt.bfloat16

B, S, D = 4, 2048, 1024
H, HD = 16, 64
SH = S // 2          # query tokens per core
NCORES = 8
PAIRS = H // 2       # head pairs, processed 2 heads at a time
DC = D // 128        # contraction chunks of 128
ST = S // 128        # key-token tiles of 128
QB = SH // 512       # query blocks of 512
KC = S // 128        # key chunks of 128
SCALE = 1.0 / np.sqrt(HD)


def _pbcast(ap, parts):
    """[1, N] AP -> [parts, N] AP with partition stride 0 (DMA broadcast)."""
    return bass.AP(tensor=ap.tensor, offset=ap.offset,
                   ap=[[0, parts]] + list(ap.ap[1:]))


def _pbcast1d(ap1d, parts):
    """[N] AP -> [parts, N] AP with partition stride 0 (DMA broadcast)."""
    return bass.AP(tensor=ap1d.tensor, offset=ap1d.offset,
                   ap=[[0, parts]] + list(ap1d.ap))


def build_graph(nc, tc, ctx):
    xt_d = nc.dram_tensor("XT", [D, S], F32, kind="ExternalInput")
    wqkv_d = nc.dram_tensor("Wqkv", [D, 3 * D], F32, kind="ExternalInput")
    bqkv_d = nc.dram_tensor("bqkv", [3 * D], F32, kind="ExternalInput")
    wproj_d = nc.dram_tensor("Wproj", [D, D], F32, kind="ExternalInput")
    bproj_d = nc.dram_tensor("bproj", [D], F32, kind="ExternalInput")
    out_d = nc.dram_tensor("out", [SH, D], F32, kind="ExternalOutput")

    const = ctx.enter_context(tc.tile_pool(name="const", bufs=1))
    stage = ctx.enter_context(tc.tile_pool(name="stage", bufs=2))
    xtp = ctx.enter_context(tc.tile_pool(name="xtp", bufs=1))
    wvp = ctx.enter_context(tc.tile_pool(name="wvp", bufs=1))
    wpairp = ctx.enter_context(tc.tile_pool(name="wpairp", bufs=2))
    qktp = ctx.enter_context(tc.tile_pool(name="qktp", bufs=2))
    vop = ctx.enter_context(tc.tile_pool(name="vop", bufs=1))
    ptp = ctx.enter_context(tc.tile_pool(name="ptp", bufs=8))
    otp = ctx.enter_context(tc.tile_pool(name="otp", bufs=1))
    recp = ctx.enter_context(tc.tile_pool(name="recp", bufs=2))
    dramp = ctx.enter_context(tc.tile_pool(name="dramp", bufs=4, space="DRAM"))
    outp = ctx.enter_context(tc.tile_pool(name="outp", bufs=2))
    psum = ctx.enter_context(tc.tile_pool(name="psum", bufs=1, space="PSUM"))

    # ---- biases -------------------------------------------------------
    # bq_cols[:, c] = bqkv[128c : 128c+128]  (per-partition bias per row-block)
    bq_cols = const.tile([128, 24], F32)
    nc.sync.dma_start(out=bq_cols, in_=bqkv_d.ap().rearrange("(c p) -> p c", p=128))
    # V bias / proj bias broadcast along partitions
    bv_bcast = const.tile([128, D], F32)
    nc.sync.dma_start(out=bv_bcast, in_=_pbcast1d(bqkv_d.ap()[2 * D:3 * D], 128))
    bp_bcast = const.tile([128, D], F32)
    nc.sync.dma_start(out=bp_bcast, in_=_pbcast1d(bproj_d.ap(), 128))

    # ---- X^T load + cast to bf16 -------------------------------------
    xt_bf = []
    for dc in range(DC):
        xst = stage.tile([128, S], F32, tag="xst", bufs=2)
        nc.sync.dma_start(out=xst, in_=xt_d.ap()[128 * dc:128 * (dc + 1), :])
        xbf = xtp.tile([128, S], BF16, tag=f"xt{dc}")
        nc.scalar.copy(xbf, xst)
        xt_bf.append(xbf)

    # ---- V = X @ W_v + b_v, stored as VO[st] = [128, H, HD+1] --------
    # (per-head 65-wide lhsT blocks: 64 v columns + a ones column that
    # computes the softmax denominator inside the PV matmul)
    wv_bf = []
    for dc in range(DC):
        wst = stage.tile([128, D], F32, tag="wst", bufs=2)
        nc.sync.dma_start(out=wst, in_=wqkv_d.ap()[128 * dc:128 * (dc + 1), 2 * D:3 * D])
        wbf = wvp.tile([128, D], BF16, tag=f"wv{dc}", bufs=1)
        nc.scalar.copy(wbf, wst)
        wv_bf.append(wbf)

    vo = []
    for st in range(ST):
        vps = psum.tile([128, D], F32, tag="big", bufs=3)
        for dc in range(DC):
            for nb in range(2):
                nc.tensor.matmul(
                    vps[:, 512 * nb:512 * (nb + 1)],
                    xt_bf[dc][:, 128 * st:128 * (st + 1)],
                    wv_bf[dc][:, 512 * nb:512 * (nb + 1)],
                    start=(dc == 0),
                    stop=(dc == DC - 1),
                )
        vt = vop.tile([128, H, HD + 1], BF16, tag=f"vo{st}")
        nc.vector.tensor_add(
            vt[:, :, 0:HD],
            vps.rearrange("p (h e) -> p h e", h=H),
            bv_bcast.rearrange("p (h e) -> p h e", h=H),
        )
        nc.vector.memset(vt[:, :, HD:HD + 1], 1.0)
        vo.append(vt)

    # ---- W_proj load + cast, reusing the wv slots (freed after V) ----
    wproj_bf = []
    for dc in range(DC):
        wst = stage.tile([128, D], F32, tag="wst", bufs=2)
        nc.sync.dma_start(out=wst, in_=wproj_d.ap()[128 * dc:128 * (dc + 1), :])
        wbf = wvp.tile([128, D], BF16, tag=f"wv{dc}", bufs=1)
        nc.vector.tensor_copy(wbf, wst)
        wproj_bf.append(wbf)

    # ---- per head-pair: Q^T, K^T then attention ----------------------
    ot = []  # per-pair attention output, transposed: [128 rows = 2*HD, SH]
    for hp in range(PAIRS):
        # W slices for this pair: cols 0:128 = Q slice, 128:256 = K slice
        wpair = []
        for dc in range(DC):
            wst = stage.tile([128, 256], F32, tag="wpst", bufs=4)
            nc.sync.dma_start(
                out=wst[:, 0:128],
                in_=wqkv_d.ap()[128 * dc:128 * (dc + 1), 128 * hp:128 * (hp + 1)],
            )
            nc.sync.dma_start(
                out=wst[:, 128:256],
                in_=wqkv_d.ap()[128 * dc:128 * (dc + 1), D + 128 * hp:D + 128 * (hp + 1)],
            )
            wbf = wpairp.tile([128, 256], BF16, tag=f"wqk{dc}", bufs=2)
            nc.vector.tensor_copy(wbf, wst)
            wpair.append(wbf)

        # Q^T for this pair: [128 rows, SH]  (rows: head0 0:64, head1 64:128)
        qt = qktp.tile([128, SH], BF16, tag="qt", bufs=2)
        qps = psum.tile([128, SH], F32, tag="big", bufs=3)
        for dc in range(DC):
            for nb in range(QB):
                nc.tensor.matmul(
                    qps[:, 512 * nb:512 * (nb + 1)],
                    wpair[dc][:, 0:128],
                    xt_bf[dc][:, 512 * nb:512 * (nb + 1)],
                    start=(dc == 0),
                    stop=(dc == DC - 1),
                )
        nc.vector.tensor_scalar_add(qt, qps, bq_cols[:, hp:hp + 1])

        # K^T for this pair: [128 rows, S]
        kt = qktp.tile([128, S], BF16, tag="kt", bufs=2)
        for kb in range(2):
            kps = psum.tile([128, SH], F32, tag="big", bufs=3)
            for dc in range(DC):
                for nb in range(2):
                    nc.tensor.matmul(
                        kps[:, 512 * nb:512 * (nb + 1)],
                        wpair[dc][:, 128:256],
                        xt_bf[dc][:, SH * kb + 512 * nb:SH * kb + 512 * (nb + 1)],
                        start=(dc == 0),
                        stop=(dc == DC - 1),
                    )
            nc.vector.tensor_scalar_add(
                kt[:, SH * kb:SH * (kb + 1)], kps, bq_cols[:, 8 + hp:9 + hp]
            )

        # attention for this pair, one 512-query block at a time
        ott = otp.tile([128, SH], BF16, tag=f"ot{hp}")
        for qb in range(QB):
            pv = [
                psum.tile([128, 512], F32, tag=f"pv{h}", bufs=1, name=f"pv{h}")
                for h in range(2)
            ]
            for kc in range(KC):
                scps = psum.tile([128, 1024], F32, tag="big", bufs=3)
                for h in range(2):
                    nc.tensor.matmul(
                        scps[:, 512 * h:512 * (h + 1)],
                        kt[64 * h:64 * (h + 1), 128 * kc:128 * (kc + 1)],
                        qt[64 * h:64 * (h + 1), 512 * qb:512 * (qb + 1)],
                        start=True,
                        stop=True,
                    )
                pt = ptp.tile([128, 1024], BF16, tag="pt", bufs=8)
                nc.scalar.activation(pt, scps, mybir.ActivationFunctionType.Exp,
                                     scale=SCALE)
                for h in range(2):
                    nc.tensor.matmul(
                        pv[h][0:HD + 1, :],
                        vo[kc][:, 2 * hp + h, :],
                        pt[:, 512 * h:512 * (h + 1)],
                        start=(kc == 0),
                        stop=(kc == KC - 1),
                    )
            for h in range(2):
                # sum_k exp lives in psum row HD; DMA-broadcast it to 64
                # partitions (DVE cannot cross partitions), then divide.
                sums = recp.tile([HD + 1, 512], F32, tag=f"sums{h}", bufs=2)
                nc.vector.tensor_copy(sums[HD:HD + 1, :], pv[h][HD:HD + 1, :])
                # partition-broadcast via a DRAM bounce (SBUF APs cannot
                # have partition stride 0, DRAM APs can)
                sumd = dramp.tile([1, 512], F32, tag=f"sumd{h}", bufs=4,
                                  name=f"sumd{h}")
                nc.sync.dma_start(out=sumd, in_=sums[HD:HD + 1, :])
                sumb = recp.tile([64, 512], F32, tag=f"sumb{h}", bufs=2)
                nc.sync.dma_start(out=sumb, in_=_pbcast(sumd, 64))
                recb = recp.tile([64, 512], F32, tag=f"recb{h}", bufs=2)
                nc.vector.reciprocal(recb, sumb)
                if h == 0:
                    nc.vector.tensor_mul(
                        ott[0:64, 512 * qb:512 * (qb + 1)], pv[h][0:HD, :], recb
                    )
                else:
                    # head1 rows live at partitions 64:128 of ott; DVE is
                    # partition-locked, so compute at base 0 and DMA-shift.
                    otmp = recp.tile([64, 512], BF16, tag="otmp", bufs=2)
                    nc.vector.tensor_mul(otmp, pv[h][0:HD, :], recb)
                    nc.sync.dma_start(
                        out=ott[64:128, 512 * qb:512 * (qb + 1)], in_=otmp
                    )
        ot.append(ott)

    # ---- output projection -------------------------------------------
    for qi in range(SH // 128):
        pps = psum.tile([128, D], F32, tag="big", bufs=3)
        for hp in range(PAIRS):
            for nb in range(2):
                nc.tensor.matmul(
                    pps[:, 512 * nb:512 * (nb + 1)],
                    ot[hp][:, 128 * qi:128 * (qi + 1)],
                    wproj_bf[hp][:, 512 * nb:512 * (nb + 1)],
                    start=(hp == 0),
                    stop=(hp == PAIRS - 1),
                )
        ost = outp.tile([128, D], F32, tag="ost", bufs=2)
        nc.vector.tensor_add(ost, pps, bp_bcast)
        nc.sync.dma_start(out=out_d.ap()[128 * qi:128 * (qi + 1), :], in_=ost)


def build_nc():
    from contextlib import ExitStack

    nc = bacc.Bacc("TRN2", target_bir_lowering=False, debug=False,
                   num_devices=NCORES)
    with tile.TileContext(nc) as tc:
        with ExitStack() as ctx:
            build_graph(nc, tc, ctx)
    nc.compile()
    return nc


def make_in_maps(X, W_qkv, b_qkv, W_proj, b_proj):
    X = np.asarray(X, dtype=np.float32)
    wqkv = np.ascontiguousarray(np.asarray(W_qkv, dtype=np.float32))
    bqkv = np.ascontiguousarray(np.asarray(b_qkv, dtype=np.float32))
    wproj = np.ascontiguousarray(np.asarray(W_proj, dtype=np.float32))
    bproj = np.ascontiguousarray(np.asarray(b_proj, dtype=np.float32))
    xts = [np.ascontiguousarray(X[b].T) for b in range(B)]
    in_maps = []
    for i in range(NCORES):
        b, h = divmod(i, 2)
        xt = xts[b] if h == 0 else np.ascontiguousarray(
            np.roll(xts[b], -SH, axis=1)
        )
        in_maps.append({
            "XT": xt, "Wqkv": wqkv, "bqkv": bqkv,
            "Wproj": wproj, "bproj": bproj,
        })
    return in_maps


_NC_CACHE = None


def get_nc():
    global _NC_CACHE
    if _NC_CACHE is None:
        _NC_CACHE = build_nc()
    return _NC_CACHE


def kernel(X, W_qkv, b_qkv, W_proj, b_proj):
    nc = get_nc()
    in_maps = make_in_maps(X, W_qkv, b_qkv, W_proj, b_proj)
    res = run_bass_kernel_spmd(nc, in_maps, core_ids=list(range(NCORES)))
    out = np.empty((B, S, D), np.float32)
    for i in range(NCORES):
        b, h = divmod(i, 2)
        out[b, h * SH:(h + 1) * SH] = res.results[i]["out"]
    return out


# revision 8
# speedup vs baseline: 1.1310x; 1.1310x over previous
"""Multi-head attention block on 8 TRN2 NeuronCores.

Sharding: data-parallel over (batch b, query-half h) -> core i = 2*b + h.
Each core computes Q for its 1024 query tokens and K/V for the full 2048-token
sequence of its batch locally (duplicated across the pair), so no collectives
are needed.

Per-core layout choices:
- X is passed host-transposed as XT = X[b].T [D, S] f32, rolled so the core's
  own query tokens occupy columns 0:1023 (attention is permutation-invariant
  over key positions, so the roll is harmless for K/V).
- All matmuls run in bf16 (1 cyc/row on PE vs 4 for fp32), accumulating f32
  in PSUM.
- Scores are computed transposed (scoresT[k, q] = K @ Q^T) so softmax exp is a
  single ACT pass per key-chunk and the denominator falls out of the PV matmul
  via a ones column appended to V ([v_h | 1] lhsT -> psum row 64 = sum_k exp).
"""

import numpy as np

import concourse.bass as bass
import concourse.tile as tile
from concourse import bacc, mybir
from concourse.bass_utils import run_bass_kernel_spmd

F32 = mybir.dt.float32
BF16 = mybir.dt.bfloat16

B, S, D = 4, 2048, 1024
H, HD = 16, 64
SH = S // 2          # query tokens per core
NCORES = 8
PAIRS = H // 2       # head pairs, processed 2 heads at a time
DC = D // 128        # contraction chunks of 128
ST = S // 128        # key-token tiles of 128
QB = SH // 512       # query blocks of 512
KC = S // 128        # key chunks of 128
SCALE = 1.0 / np.sqrt(HD)


def _pbcast(ap, parts):
    """[1, N] AP -> [parts, N] AP with partition stride 0 (DMA broadcast)."""
    return bass.AP(tensor=ap.tensor, offset=ap.offset,
                   ap=[[0, parts]] + list(ap.ap[1:]))


def _pbcast1d(ap1d, parts):
    """[N] AP -> [parts, N] AP with partition stride 0 (DMA broadcast)."""
    return bass.AP(tensor=ap1d.tensor, offset=ap1d.offset,
                   ap=[[0, parts]] + list(ap1d.ap))


def build_graph(nc, tc, ctx):
    xt_d = nc.dram_tensor("XT", [D, S], F32, kind="ExternalInput")
    wqkv_d = nc.dram_tensor("Wqkv", [D, 3 * D], F32, kind="ExternalInput")
    bqkv_d = nc.dram_tensor("bqkv", [3 * D], F32, kind="ExternalInput")
    wproj_d = nc.dram_tensor("Wproj", [D, D], F32, kind="ExternalInput")
    bproj_d = nc.dram_tensor("bproj", [D], F32, kind="ExternalInput")
    out_d = nc.dram_tensor("out", [SH, D], F32, kind="ExternalOutput")

    const = ctx.enter_context(tc.tile_pool(name="const", bufs=1))
    stage = ctx.enter_context(tc.tile_pool(name="stage", bufs=2))
    xtp = ctx.enter_context(tc.tile_pool(name="xtp", bufs=1))
    wvp = ctx.enter_context(tc.tile_pool(name="wvp", bufs=1))
    wpairp = ctx.enter_context(tc.tile_pool(name="wpairp", bufs=2))
    qktp = ctx.enter_context(tc.tile_pool(name="qktp", bufs=2))
    vop = ctx.enter_context(tc.tile_pool(name="vop", bufs=1))
    ptp = ctx.enter_context(tc.tile_pool(name="ptp", bufs=8))
    otp = ctx.enter_context(tc.tile_pool(name="otp", bufs=1))
    recp = ctx.enter_context(tc.tile_pool(name="recp", bufs=2))
    dramp = ctx.enter_context(tc.tile_pool(name="dramp", bufs=4, space="DRAM"))
    outp = ctx.enter_context(tc.tile_pool(name="outp", bufs=2))
    psum = ctx.enter_context(tc.tile_pool(name="psum", bufs=1, space="PSUM"))

    # ---- biases -------------------------------------------------------
    # bq_cols[:, c] = bqkv[128c : 128c+128]  (per-partition bias per row-block)
    bq_cols = const.tile([128, 24], F32)
    nc.sync.dma_start(out=bq_cols, in_=bqkv_d.ap().rearrange("(c p) -> p c", p=128))
    # V bias / proj bias broadcast along partitions
    bv_bcast = const.tile([128, D], F32)
    nc.sync.dma_start(out=bv_bcast, in_=_pbcast1d(bqkv_d.ap()[2 * D:3 * D], 128))
    bp_bcast = const.tile([128, D], F32)
    nc.sync.dma_start(out=bp_bcast, in_=_pbcast1d(bproj_d.ap(), 128))

    # ---- X^T load + cast to bf16 -------------------------------------
    xt_bf = []
    for dc in range(DC):
        xst = stage.tile([128, S], F32, tag="xst", bufs=2)
        nc.sync.dma_start(out=xst, in_=xt_d.ap()[128 * dc:128 * (dc + 1), :])
        xbf = xtp.tile([128, S], BF16, tag=f"xt{dc}")
        nc.vector.tensor_copy(xbf, xst)
        xt_bf.append(xbf)

    # ---- V = X @ W_v + b_v, stored as VO[st] = [128, H, HD+1] --------
    # (per-head 65-wide lhsT blocks: 64 v columns + a ones column that
    # computes the softmax denominator inside the PV matmul)
    wv_bf = []
    for dc in range(DC):
        wst = stage.tile([128, D], F32, tag="wst", bufs=2)
        nc.sync.dma_start(out=wst, in_=wqkv_d.ap()[128 * dc:128 * (dc + 1), 2 * D:3 * D])
        wbf = wvp.tile([128, D], BF16, tag=f"wv{dc}", bufs=1)
        nc.vector.tensor_copy(wbf, wst)
        wv_bf.append(wbf)

    vo = []
    for st in range(ST):
        vps = psum.tile([128, D], F32, tag="big", bufs=2)
        for dc in range(DC):
            for nb in range(2):
                nc.tensor.matmul(
                    vps[:, 512 * nb:512 * (nb + 1)],
                    xt_bf[dc][:, 128 * st:128 * (st + 1)],
                    wv_bf[dc][:, 512 * nb:512 * (nb + 1)],
                    start=(dc == 0),
                    stop=(dc == DC - 1),
                )
        vt = vop.tile([128, H, HD + 1], BF16, tag=f"vo{st}")
        nc.vector.tensor_add(
            vt[:, :, 0:HD],
            vps.rearrange("p (h e) -> p h e", h=H),
            bv_bcast.rearrange("p (h e) -> p h e", h=H),
        )
        nc.vector.memset(vt[:, :, HD:HD + 1], 1.0)
        vo.append(vt)

    # ---- W_proj load + cast, reusing the wv slots (freed after V) ----
    wproj_bf = []
    for dc in range(DC):
        wst = stage.tile([128, D], F32, tag="wst", bufs=2)
        nc.sync.dma_start(out=wst, in_=wproj_d.ap()[128 * dc:128 * (dc + 1), :])
        wbf = wvp.tile([128, D], BF16, tag=f"wv{dc}", bufs=1)
        nc.vector.tensor_copy(wbf, wst)
        wproj_bf.append(wbf)

    # ---- per head-pair: Q^T, K^T then attention ----------------------
    ot = []  # per-pair attention output, transposed: [128 rows = 2*HD, SH]
    for hp in range(PAIRS):
        # W slices for this pair: cols 0:128 = Q slice, 128:256 = K slice
        wpair = []
        for dc in range(DC):
            wst = stage.tile([128, 256], F32, tag="wpst", bufs=4)
            nc.sync.dma_start(
                out=wst[:, 0:128],
                in_=wqkv_d.ap()[128 * dc:128 * (dc + 1), 128 * hp:128 * (hp + 1)],
            )
            nc.sync.dma_start(
                out=wst[:, 128:256],
                in_=wqkv_d.ap()[128 * dc:128 * (dc + 1), D + 128 * hp:D + 128 * (hp + 1)],
            )
            wbf = wpairp.tile([128, 256], BF16, tag=f"wqk{dc}", bufs=2)
            nc.vector.tensor_copy(wbf, wst)
            wpair.append(wbf)

        # Q^T for this pair: [128 rows, SH]  (rows: head0 0:64, head1 64:128)
        qt = qktp.tile([128, SH], BF16, tag="qt", bufs=2)
        qps = psum.tile([128, SH], F32, tag="big", bufs=2)
        for dc in range(DC):
            for nb in range(QB):
                nc.tensor.matmul(
                    qps[:, 512 * nb:512 * (nb + 1)],
                    wpair[dc][:, 0:128],
                    xt_bf[dc][:, 512 * nb:512 * (nb + 1)],
                    start=(dc == 0),
                    stop=(dc == DC - 1),
                )
        nc.vector.tensor_scalar_add(qt, qps, bq_cols[:, hp:hp + 1])

        # K^T for this pair: [128 rows, S]
        kt = qktp.tile([128, S], BF16, tag="kt", bufs=2)
        for kb in range(2):
            kps = psum.tile([128, SH], F32, tag="big", bufs=2)
            for dc in range(DC):
                for nb in range(2):
                    nc.tensor.matmul(
                        kps[:, 512 * nb:512 * (nb + 1)],
                        wpair[dc][:, 128:256],
                        xt_bf[dc][:, SH * kb + 512 * nb:SH * kb + 512 * (nb + 1)],
                        start=(dc == 0),
                        stop=(dc == DC - 1),
                    )
            nc.vector.tensor_scalar_add(
                kt[:, SH * kb:SH * (kb + 1)], kps, bq_cols[:, 8 + hp:9 + hp]
            )

        # attention for this pair, one 512-query block at a time
        ott = otp.tile([128, SH], BF16, tag=f"ot{hp}")
        for qb in range(QB):
            pv = [
                psum.tile([128, 512], F32, tag=f"pv{h}", bufs=2, name=f"pv{h}")
                for h in range(2)
            ]
            for kc in range(KC):
                scps = psum.tile([128, 1024], F32, tag="big", bufs=2)
                for h in range(2):
                    nc.tensor.matmul(
                        scps[:, 512 * h:512 * (h + 1)],
                        kt[64 * h:64 * (h + 1), 128 * kc:128 * (kc + 1)],
                        qt[64 * h:64 * (h + 1), 512 * qb:512 * (qb + 1)],
                        start=True,
                        stop=True,
                    )
                pt = ptp.tile([128, 1024], BF16, tag="pt", bufs=8)
                nc.scalar.activation(pt, scps, mybir.ActivationFunctionType.Exp,
                                     scale=SCALE)
                for h in range(2):
                    nc.tensor.matmul(
                        pv[h][0:HD + 1, :],
                        vo[kc][:, 2 * hp + h, :],
                        pt[:, 512 * h:512 * (h + 1)],
                        start=(kc == 0),
                        stop=(kc == KC - 1),
                    )
            for h in range(2):
                # sum_k exp lives in psum row HD; DMA-broadcast it to 64
                # partitions (DVE cannot cross partitions), then divide.
                sums = recp.tile([HD + 1, 512], F32, tag=f"sums{h}", bufs=2)
                nc.vector.tensor_copy(sums[HD:HD + 1, :], pv[h][HD:HD + 1, :])
                # partition-broadcast via a DRAM bounce (SBUF APs cannot
                # have partition stride 0, DRAM APs can)
                sumd = dramp.tile([1, 512], F32, tag=f"sumd{h}", bufs=4,
                                  name=f"sumd{h}")
                nc.sync.dma_start(out=sumd, in_=sums[HD:HD + 1, :])
                sumb = recp.tile([64, 512], F32, tag=f"sumb{h}", bufs=2)
                nc.sync.dma_start(out=sumb, in_=_pbcast(sumd, 64))
                recb = recp.tile([64, 512], F32, tag=f"recb{h}", bufs=2)
                nc.vector.reciprocal_approx_fast(recb, sumb)
                if h == 0:
                    nc.vector.tensor_mul(
                        ott[0:64, 512 * qb:512 * (qb + 1)], pv[h][0:HD, :], recb
                    )
                else:
                    # head1 rows live at partitions 64:128 of ott; DVE is
                    # partition-locked, so compute at base 0 and DMA-shift.
                    otmp = recp.tile([64, 512], BF16, tag="otmp", bufs=2)
                    nc.vector.tensor_mul(otmp, pv[h][0:HD, :], recb)
                    nc.sync.dma_start(
                        out=ott[64:128, 512 * qb:512 * (qb + 1)], in_=otmp
                    )
        ot.append(ott)

    # ---- output projection -------------------------------------------
    for qi in range(SH // 128):
        pps = psum.tile([128, D], F32, tag="big", bufs=2)
        for hp in range(PAIRS):
            for nb in range(2):
                nc.tensor.matmul(
                    pps[:, 512 * nb:512 * (nb + 1)],
                    ot[hp][:, 128 * qi:128 * (qi + 1)],
                    wproj_bf[hp][:, 512 * nb:512 * (nb + 1)],
                    start=(hp == 0),
                    stop=(hp == PAIRS - 1),
                )
        ost = outp.tile([128, D], F32, tag="ost", bufs=2)
        nc.vector.tensor_add(ost, pps, bp_bcast)
        nc.sync.dma_start(out=out_d.ap()[128 * qi:128 * (qi + 1), :], in_=ost)


def build_nc():
    from contextlib import ExitStack

    nc = bacc.Bacc("TRN2", target_bir_lowering=False, debug=False,
                   num_devices=NCORES)
    with tile.TileContext(nc) as tc:
        with ExitStack() as ctx:
            build_graph(nc, tc, ctx)
    nc.compile()
    return nc


def make_in_maps(X, W_qkv, b_qkv, W_proj, b_proj):
    X = np.asarray(X, dtype=np.float32)
    wqkv = np.ascontiguousarray(np.asarray(W_qkv, dtype=np.float32))
    bqkv = np.ascontiguousarray(np.asarray(b_qkv, dtype=np.float32))
    wproj = np.ascontiguousarray(np.asarray(W_proj, dtype=np.float32))
    bproj = np.ascontiguousarray(np.asarray(b_proj, dtype=np.float32))
    xts = [np.ascontiguousarray(X[b].T) for b in range(B)]
    in_maps = []
    for i in range(NCORES):
        b, h = divmod(i, 2)
        xt = xts[b] if h == 0 else np.ascontiguousarray(
            np.roll(xts[b], -SH, axis=1)
        )
        in_maps.append({
            "XT": xt, "Wqkv": wqkv, "bqkv": bqkv,
            "Wproj": wproj, "bproj": bproj,
        })
    return in_maps


_NC_CACHE = None


def get_nc():
    global _NC_CACHE
    if _NC_CACHE is None:
        _NC_CACHE = build_nc()
    return _NC_CACHE


def kernel(X, W_qkv, b_qkv, W_proj, b_proj):
    nc = get_nc()
    in_maps = make_in_maps(X, W_qkv, b_qkv, W_proj, b_proj)
    res = run_bass_kernel_spmd(nc, in_maps, core_ids=list(range(NCORES)))
    out = np.empty((B, S, D), np.float32)
    for i in range(NCORES):
        b, h = divmod(i, 2)
        out[b, h * SH:(h + 1) * SH] = res.results[i]["out"]
    return out


# revision 13
# speedup vs baseline: 1.1547x; 1.0210x over previous
"""Multi-head attention block on 8 TRN2 NeuronCores.

Sharding: data-parallel over (batch b, query-half h) -> core i = 2*b + h.
Each core computes Q for its 1024 query tokens and K/V for the full 2048-token
sequence of its batch locally (duplicated across the pair), so no collectives
are needed.

Per-core layout choices:
- X is passed host-transposed as XT = X[b].T [D, S] f32, rolled so the core's
  own query tokens occupy columns 0:1023 (attention is permutation-invariant
  over key positions, so the roll is harmless for K/V).
- All matmuls run in bf16 (1 cyc/row on PE vs 4 for fp32), accumulating f32
  in PSUM.
- Scores are computed transposed (scoresT[k, q] = K @ Q^T) so softmax exp is a
  single ACT pass per key-chunk and the denominator falls out of the PV matmul
  via a ones column appended to V ([v_h | 1] lhsT -> psum row 64 = sum_k exp).
- The V projection and the next pair's Q/K projections are emitted interleaved
  into the attention key-chunk loop, so the (in-order) PE stream has no
  head-of-line stalls waiting on the exp stream and the ACT engine never
  starves at pair boundaries.
"""

import numpy as np

import concourse.bass as bass
import concourse.tile as tile
from concourse import bacc, mybir
from concourse.bass_utils import run_bass_kernel_spmd

F32 = mybir.dt.float32
F32R = mybir.dt.float32r
BF16 = mybir.dt.bfloat16

B, S, D = 4, 2048, 1024
H, HD = 16, 64
SH = S // 2          # query tokens per core
NCORES = 8
PAIRS = H // 2       # head pairs, processed 2 heads at a time
DC = D // 128        # contraction chunks of 128
ST = S // 128        # key-token tiles of 128
QB = SH // 512       # query blocks of 512
KC = S // 128        # key chunks of 128
SCALE = 1.0 / np.sqrt(HD)


def _pbcast1d(ap1d, parts):
    """[N] AP -> [parts, N] AP with partition stride 0 (DMA broadcast)."""
    return bass.AP(tensor=ap1d.tensor, offset=ap1d.offset,
                   ap=[[0, parts]] + list(ap1d.ap))


def build_graph(nc, tc, ctx):
    xt_d = nc.dram_tensor("XT", [D, S], F32, kind="ExternalInput")
    wqkv_d = nc.dram_tensor("Wqkv", [D, 3 * D], F32, kind="ExternalInput")
    bqkv_d = nc.dram_tensor("bqkv", [3 * D], F32, kind="ExternalInput")
    wproj_d = nc.dram_tensor("Wproj", [D, D], F32, kind="ExternalInput")
    bproj_d = nc.dram_tensor("bproj", [D], F32, kind="ExternalInput")
    out_d = nc.dram_tensor("out", [SH, D], F32, kind="ExternalOutput")

    const = ctx.enter_context(tc.tile_pool(name="const", bufs=1))
    stage = ctx.enter_context(tc.tile_pool(name="stage", bufs=2))
    xtp = ctx.enter_context(tc.tile_pool(name="xtp", bufs=1))
    wvp = ctx.enter_context(tc.tile_pool(name="wvp", bufs=1))
    wpairp = ctx.enter_context(tc.tile_pool(name="wpairp", bufs=2))
    qktp = ctx.enter_context(tc.tile_pool(name="qktp", bufs=2))
    vop = ctx.enter_context(tc.tile_pool(name="vop", bufs=1))
    ptp = ctx.enter_context(tc.tile_pool(name="ptp", bufs=8))
    otp = ctx.enter_context(tc.tile_pool(name="otp", bufs=1))
    recp = ctx.enter_context(tc.tile_pool(name="recp", bufs=2))
    outp = ctx.enter_context(tc.tile_pool(name="outp", bufs=2))
    psum = ctx.enter_context(tc.tile_pool(name="psum", bufs=1, space="PSUM"))

    def big_psum(name):
        return psum.tile([128, 1024], F32, tag="big", bufs=3, name=name)

    # ---- X^T load + cast to bf16, W_v interleaved --------------------
    xt_st = []
    for dc in range(DC):
        xst = stage.tile([128, S], F32, tag="xst", bufs=3, name="xst")
        nc.sync.dma_start(out=xst, in_=xt_d.ap()[128 * dc:128 * (dc + 1), :])
        xt_st.append(xst)
    wv_bf = []
    for dc in range(DC):
        wst = stage.tile([128, D], F32, tag="wst", bufs=2, name="wst")
        nc.sync.dma_start(out=wst,
                          in_=wqkv_d.ap()[128 * dc:128 * (dc + 1), 2 * D:3 * D])
        wbf = wvp.tile([128, D], BF16, tag=f"wv{dc}", bufs=1, name=f"wv{dc}")
        nc.vector.tensor_copy(wbf, wst)
        wv_bf.append(wbf)
    xt_bf = []
    for dc in range(DC):
        xbf = xtp.tile([128, S], BF16, tag=f"xt{dc}", name=f"xt{dc}")
        nc.vector.tensor_copy(xbf, xt_st[dc])
        xt_bf.append(xbf)

    # ---- biases (gpsimd queue so the gather doesn't block big loads) --
    bq_cols = const.tile([128, 24], F32)
    nc.gpsimd.dma_start(out=bq_cols,
                        in_=bqkv_d.ap().rearrange("(c p) -> p c", p=128))
    bv_bcast = const.tile([128, D], F32)
    nc.gpsimd.dma_start(out=bv_bcast, in_=_pbcast1d(bqkv_d.ap()[2 * D:3 * D], 128))
    bp_bcast = const.tile([128, D], F32)
    nc.gpsimd.dma_start(out=bp_bcast, in_=_pbcast1d(bproj_d.ap(), 128))
    ones_f = const.tile([HD + 1, HD], F32)
    nc.vector.memset(ones_f, 1.0)
    ones_bc = const.tile([HD + 1, HD], F32R)
    nc.vector.tensor_copy(ones_bc, ones_f)

    # ---- deferred-emission pieces ------------------------------------
    vo = [None] * ST

    def emit_v(st):
        """V = X @ W_v + b_v for one 128-token tile, as [128, H, HD+1]."""
        vps = big_psum(f"vps{st}")
        for dc in range(DC):
            for nb in range(2):
                nc.tensor.matmul(
                    vps[:, 512 * nb:512 * (nb + 1)],
                    xt_bf[dc][:, 128 * st:128 * (st + 1)],
                    wv_bf[dc][:, 512 * nb:512 * (nb + 1)],
                    start=(dc == 0),
                    stop=(dc == DC - 1),
                )
        vt = vop.tile([128, H, HD + 1], BF16, tag=f"vo{st}", name=f"vo{st}")
        nc.vector.tensor_add(
            vt[:, :, 0:HD],
            vps.rearrange("p (h e) -> p h e", h=H),
            bv_bcast.rearrange("p (h e) -> p h e", h=H),
        )
        nc.vector.memset(vt[:, :, HD:HD + 1], 1.0)
        vo[st] = vt

    def emit_wpair(hp):
        """DMA + cast of this pair's W_q / W_k column slices (no PE work)."""
        wpair = []
        for dc in range(DC):
            wst = stage.tile([128, 256], F32, tag="wpst", bufs=4, name="wpst")
            nc.sync.dma_start(
                out=wst[:, 0:128],
                in_=wqkv_d.ap()[128 * dc:128 * (dc + 1), 128 * hp:128 * (hp + 1)],
            )
            nc.sync.dma_start(
                out=wst[:, 128:256],
                in_=wqkv_d.ap()[128 * dc:128 * (dc + 1),
                                D + 128 * hp:D + 128 * (hp + 1)],
            )
            wbf = wpairp.tile([128, 256], BF16, tag=f"wqk{dc}", bufs=2,
                              name=f"wqk{dc}")
            nc.vector.tensor_copy(wbf, wst)
            wpair.append(wbf)
        return wpair

    def qk_pieces(hp, wpair, sink):
        """Return a list of closures, each emitting one PE matmul (plus the
        occasional DVE bias-copy) of pair hp's Q^T/K^T projections. Executing
        all of them fills sink dict with qt/kt tiles."""
        qt = qktp.tile([128, SH], BF16, tag="qt", bufs=2, name=f"qt{hp}")
        kt = qktp.tile([128, S], BF16, tag="kt", bufs=2, name=f"kt{hp}")
        sink["qt"], sink["kt"] = qt, kt
        pieces = []
        holder = {}

        def q_mm(dc, nb):
            def f():
                if "qps" not in holder:
                    holder["qps"] = big_psum(f"qps{hp}")
                nc.tensor.matmul(
                    holder["qps"][:, 512 * nb:512 * (nb + 1)],
                    wpair[dc][:, 0:128],
                    xt_bf[dc][:, 512 * nb:512 * (nb + 1)],
                    start=(dc == 0),
                    stop=(dc == DC - 1),
                )
                if dc == DC - 1 and nb == QB - 1:
                    nc.vector.tensor_scalar_add(qt, holder["qps"],
                                                bq_cols[:, hp:hp + 1])
            return f

        def k_mm(kb, dc, nb):
            def f():
                key = f"kps{kb}"
                if key not in holder:
                    holder[key] = big_psum(f"kps{hp}_{kb}")
                nc.tensor.matmul(
                    holder[key][:, 512 * nb:512 * (nb + 1)],
                    wpair[dc][:, 128:256],
                    xt_bf[dc][:, SH * kb + 512 * nb:SH * kb + 512 * (nb + 1)],
                    start=(dc == 0),
                    stop=(dc == DC - 1),
                )
                if dc == DC - 1 and nb == 1:
                    nc.vector.tensor_scalar_add(
                        kt[:, SH * kb:SH * (kb + 1)], holder[key],
                        bq_cols[:, 8 + hp:9 + hp])
            return f

        for dc in range(DC):
            for nb in range(QB):
                pieces.append(q_mm(dc, nb))
        for kb in range(2):
            for dc in range(DC):
                for nb in range(2):
                    pieces.append(k_mm(kb, dc, nb))
        return pieces

    def attention_pair(hp, qt, kt, fillers):
        """Attention for head pair hp. fillers: per-step closures emitting
        unrelated PE work (V tiles / next pair's QK matmuls) so the PE
        stream stays dense while ACT paces the exp chain."""
        nfill = len(fillers)
        fi = 0
        ott = otp.tile([128, SH], BF16, tag=f"ot{hp}", name=f"ot{hp}")
        for qb in range(QB):
            pv = [
                psum.tile([128, 512], F32, tag=f"pv{h}", bufs=1, name=f"pv{h}")
                for h in range(2)
            ]
            for kc in range(KC):
                step = qb * KC + kc
                # V for this key chunk must exist before its PV matmul
                if hp == 0 and qb == 0:
                    emit_v(kc)
                if hp == 0:
                    # qb0 already carries the V emission; fillers go in qb1
                    want = 0 if qb == 0 else (kc + 1) * nfill // KC
                else:
                    want = (step + 1) * nfill // (2 * KC)
                while fi < want:
                    fillers[fi]()
                    fi += 1
                scps = big_psum(f"sc{hp}_{qb}_{kc}")
                for h in range(2):
                    nc.tensor.matmul(
                        scps[:, 512 * h:512 * (h + 1)],
                        kt[64 * h:64 * (h + 1), 128 * kc:128 * (kc + 1)],
                        qt[64 * h:64 * (h + 1), 512 * qb:512 * (qb + 1)],
                        start=True,
                        stop=True,
                    )
                pt = ptp.tile([128, 1024], BF16, tag="pt", bufs=8, name="pt")
                nc.scalar.activation(pt, scps, mybir.ActivationFunctionType.Exp,
                                     scale=SCALE)
                for h in range(2):
                    nc.tensor.matmul(
                        pv[h][0:HD + 1, :],
                        vo[kc][:, 2 * hp + h, :],
                        pt[:, 512 * h:512 * (h + 1)],
                        start=(kc == 0),
                        stop=(kc == KC - 1),
                    )
            # normalize: sums row -> partition-broadcast via tiny f32r
            # matmul (ones column) -> reciprocal -> scale the output rows
            bc = big_psum(f"bc{hp}_{qb}")
            for h in range(2):
                sums = recp.tile([HD + 1, 512], F32R, tag=f"sums{h}", bufs=2,
                                 name=f"sums{h}")
                nc.vector.tensor_copy(sums[HD:HD + 1, :], pv[h][HD:HD + 1, :])
                nc.tensor.matmul(
                    bc[0:HD, 512 * h:512 * (h + 1)],
                    ones_bc[HD:HD + 1, :],
                    sums[HD:HD + 1, :],
                    start=True,
                    stop=True,
                    tile_position=(64, 0),
                )
            for h in range(2):
                recb = recp.tile([64, 512], F32, tag=f"recb{h}", bufs=2,
                                 name=f"recb{h}")
                nc.vector.reciprocal_approx_fast(
                    recb, bc[0:HD, 512 * h:512 * (h + 1)])
                if h == 0:
                    nc.vector.tensor_mul(
                        ott[0:64, 512 * qb:512 * (qb + 1)], pv[h][0:HD, :], recb
                    )
                else:
                    # head1 rows live at partitions 64:128 of ott; DVE is
                    # partition-locked, so compute at base 0 and DMA-shift.
                    otmp = recp.tile([64, 512], BF16, tag="otmp", bufs=2,
                                     name="otmp")
                    nc.vector.tensor_mul(otmp, pv[h][0:HD, :], recb)
                    nc.sync.dma_start(
                        out=ott[64:128, 512 * qb:512 * (qb + 1)], in_=otmp
                    )
        return ott

    # ---- pair pipeline ------------------------------------------------
    wpair0 = emit_wpair(0)
    sink = {}
    for piece in qk_pieces(0, wpair0, sink):
        piece()

    wproj_bf = []
    ot = []
    for hp in range(PAIRS):
        qt, kt = sink["qt"], sink["kt"]
        fillers = []
        if hp < PAIRS - 1:
            wpair_n = emit_wpair(hp + 1)
            sink = {}
            fillers = qk_pieces(hp + 1, wpair_n, sink)
        if hp == 0:
            # W_proj loads ride in the wv slots, which free after the last
            # V tile; emit now so the DMA engine fills them mid-attention.
            for dc in range(DC):
                wst = stage.tile([128, D], F32, tag="wst", bufs=2, name="wst")
                nc.sync.dma_start(
                    out=wst, in_=wproj_d.ap()[128 * dc:128 * (dc + 1), :])
                wbf = wvp.tile([128, D], BF16, tag=f"wv{dc}", bufs=1,
                               name=f"wvp{dc}")
                nc.vector.tensor_copy(wbf, wst)
                wproj_bf.append(wbf)
        ot.append(attention_pair(hp, qt, kt, fillers))

    # ---- output projection -------------------------------------------
    for qi in range(SH // 128):
        pps = big_psum(f"pps{qi}")
        for hp in range(PAIRS):
            for nb in range(2):
                nc.tensor.matmul(
                    pps[:, 512 * nb:512 * (nb + 1)],
                    ot[hp][:, 128 * qi:128 * (qi + 1)],
                    wproj_bf[hp][:, 512 * nb:512 * (nb + 1)],
                    start=(hp == 0),
                    stop=(hp == PAIRS - 1),
                )
        ost = outp.tile([128, D], F32, tag="ost", bufs=2, name="ost")
        nc.vector.tensor_add(ost, pps, bp_bcast)
        nc.sync.dma_start(out=out_d.ap()[128 * qi:128 * (qi + 1), :], in_=ost)


def build_nc():
    from contextlib import ExitStack

    nc = bacc.Bacc("TRN2", target_bir_lowering=False, debug=False,
                   num_devices=NCORES)
    with tile.TileContext(nc) as tc:
        with ExitStack() as ctx:
            build_graph(nc, tc, ctx)
    nc.compile()
    return nc


def make_in_maps(X, W_qkv, b_qkv, W_proj, b_proj):
    X = np.asarray(X, dtype=np.float32)
    wqkv = np.ascontiguousarray(np.asarray(W_qkv, dtype=np.float32))
    bqkv = np.ascontiguousarray(np.asarray(b_qkv, dtype=np.float32))
    wproj = np.ascontiguousarray(np.asarray(W_proj, dtype=np.float32))
    bproj = np.ascontiguousarray(np.asarray(b_proj, dtype=np.float32))
    xts = [np.ascontiguousarray(X[b].T) for b in range(B)]
    in_maps = []
    for i in range(NCORES):
        b, h = divmod(i, 2)
        xt = xts[b] if h == 0 else np.ascontiguousarray(
            np.roll(xts[b], -SH, axis=1)
        )
        in_maps.append({
            "XT": xt, "Wqkv": wqkv, "bqkv": bqkv,
            "Wproj": wproj, "bproj": bproj,
        })
    return in_maps


_NC_CACHE = None


def get_nc():
    global _NC_CACHE
    if _NC_CACHE is None:
        _NC_CACHE = build_nc()
    return _NC_CACHE


def kernel(X, W_qkv, b_qkv, W_proj, b_proj):
    nc = get_nc()
    in_maps = make_in_maps(X, W_qkv, b_qkv, W_proj, b_proj)
    res = run_bass_kernel_spmd(nc, in_maps, core_ids=list(range(NCORES)))
    out = np.empty((B, S, D), np.float32)
    for i in range(NCORES):
        b, h = divmod(i, 2)
        out[b, h * SH:(h + 1) * SH] = res.results[i]["out"]
    return out


# revision 15
# speedup vs baseline: 1.2146x; 1.0518x over previous
"""Multi-head attention block on 8 TRN2 NeuronCores.

Sharding: data-parallel over (batch b, query-half h) -> core i = 2*b + h.
Each core computes Q for its 1024 query tokens and K/V for the full 2048-token
sequence of its batch locally (duplicated across the pair), so no collectives
are needed.

Per-core layout choices:
- X is passed host-transposed as XT = X[b].T [D, S] f32, rolled so the core's
  own query tokens occupy columns 0:1023 (attention is permutation-invariant
  over key positions, so the roll is harmless for K/V).
- All matmuls run in bf16 (1 cyc/row on PE vs 4 for fp32), accumulating f32
  in PSUM.
- Scores are computed transposed (scoresT[k, q] = K @ Q^T) so softmax exp is a
  single ACT pass per key-chunk and the denominator falls out of the PV matmul
  via a ones column appended to V ([v_h | 1] lhsT -> psum row 64 = sum_k exp).
- The V projection and the next pair's Q/K projections are emitted interleaved
  into the attention key-chunk loop, so the (in-order) PE stream has no
  head-of-line stalls waiting on the exp stream and the ACT engine never
  starves at pair boundaries.
"""

import numpy as np

import concourse.bass as bass
import concourse.tile as tile
from concourse import bacc, mybir
from concourse.bass_utils import run_bass_kernel_spmd

F32 = mybir.dt.float32
F32R = mybir.dt.float32r
BF16 = mybir.dt.bfloat16

B, S, D = 4, 2048, 1024
H, HD = 16, 64
SH = S // 2          # query tokens per core
NCORES = 8
PAIRS = H // 2       # head pairs, processed 2 heads at a time
DC = D // 128        # contraction chunks of 128
ST = S // 128        # key-token tiles of 128
QB = SH // 512       # query blocks of 512
KC = S // 128        # key chunks of 128
SCALE = 1.0 / np.sqrt(HD)


def _pbcast1d(ap1d, parts):
    """[N] AP -> [parts, N] AP with partition stride 0 (DMA broadcast)."""
    return bass.AP(tensor=ap1d.tensor, offset=ap1d.offset,
                   ap=[[0, parts]] + list(ap1d.ap))


def build_graph(nc, tc, ctx):
    xt_d = nc.dram_tensor("XT", [D, S], F32, kind="ExternalInput")
    wqkv_d = nc.dram_tensor("Wqkv", [D, 3 * D], F32, kind="ExternalInput")
    bqkv_d = nc.dram_tensor("bqkv", [3 * D], F32, kind="ExternalInput")
    wproj_d = nc.dram_tensor("Wproj", [D, D], F32, kind="ExternalInput")
    bproj_d = nc.dram_tensor("bproj", [D], F32, kind="ExternalInput")
    out_d = nc.dram_tensor("out", [SH, D], F32, kind="ExternalOutput")

    const = ctx.enter_context(tc.tile_pool(name="const", bufs=1))
    stage = ctx.enter_context(tc.tile_pool(name="stage", bufs=2))
    xtp = ctx.enter_context(tc.tile_pool(name="xtp", bufs=1))
    wvp = ctx.enter_context(tc.tile_pool(name="wvp", bufs=1))
    wpairp = ctx.enter_context(tc.tile_pool(name="wpairp", bufs=2))
    qktp = ctx.enter_context(tc.tile_pool(name="qktp", bufs=2))
    vop = ctx.enter_context(tc.tile_pool(name="vop", bufs=1))
    ptp = ctx.enter_context(tc.tile_pool(name="ptp", bufs=8))
    otp = ctx.enter_context(tc.tile_pool(name="otp", bufs=1))
    recp = ctx.enter_context(tc.tile_pool(name="recp", bufs=2))
    outp = ctx.enter_context(tc.tile_pool(name="outp", bufs=2))
    psum = ctx.enter_context(tc.tile_pool(name="psum", bufs=1, space="PSUM"))

    def big_psum(name):
        return psum.tile([128, 1024], F32, tag="big", bufs=3, name=name)

    # ---- X^T load + cast to bf16, W_v interleaved --------------------
    xt_st = []
    for dc in range(DC):
        xst = stage.tile([128, S], F32, tag="xst", bufs=3, name="xst")
        nc.sync.dma_start(out=xst, in_=xt_d.ap()[128 * dc:128 * (dc + 1), :])
        xt_st.append(xst)
    wv_bf = []
    for dc in range(DC):
        wst = stage.tile([128, D], F32, tag="wst", bufs=2, name="wst")
        nc.sync.dma_start(out=wst,
                          in_=wqkv_d.ap()[128 * dc:128 * (dc + 1), 2 * D:3 * D])
        wbf = wvp.tile([128, D], BF16, tag=f"wv{dc}", bufs=1, name=f"wv{dc}")
        nc.vector.tensor_copy(wbf, wst)
        wv_bf.append(wbf)
    xt_bf = []
    for dc in range(DC):
        xbf = xtp.tile([128, S], BF16, tag=f"xt{dc}", name=f"xt{dc}")
        nc.vector.tensor_copy(xbf, xt_st[dc])
        xt_bf.append(xbf)

    # ---- biases (gpsimd queue so the gather doesn't block big loads) --
    bq_cols = const.tile([128, 24], F32)
    nc.gpsimd.dma_start(out=bq_cols,
                        in_=bqkv_d.ap().rearrange("(c p) -> p c", p=128))
    bv_bcast = const.tile([128, D], F32)
    nc.gpsimd.dma_start(out=bv_bcast, in_=_pbcast1d(bqkv_d.ap()[2 * D:3 * D], 128))
    bp_bcast = const.tile([128, D], F32)
    nc.gpsimd.dma_start(out=bp_bcast, in_=_pbcast1d(bproj_d.ap(), 128))
    ones_f = const.tile([HD + 1, HD], F32)
    nc.vector.memset(ones_f, 1.0)
    ones_bc = const.tile([HD + 1, HD], F32R)
    nc.vector.tensor_copy(ones_bc, ones_f)

    # ---- deferred-emission pieces ------------------------------------
    vo = [None] * ST

    def emit_v(st):
        """V = X @ W_v + b_v for one 128-token tile, as [128, H, HD+1]."""
        vps = big_psum(f"vps{st}")
        for dc in range(DC):
            for nb in range(2):
                nc.tensor.matmul(
                    vps[:, 512 * nb:512 * (nb + 1)],
                    xt_bf[dc][:, 128 * st:128 * (st + 1)],
                    wv_bf[dc][:, 512 * nb:512 * (nb + 1)],
                    start=(dc == 0),
                    stop=(dc == DC - 1),
                )
        vt = vop.tile([128, H, HD + 1], BF16, tag=f"vo{st}", name=f"vo{st}")
        nc.vector.tensor_add(
            vt[:, :, 0:HD],
            vps.rearrange("p (h e) -> p h e", h=H),
            bv_bcast.rearrange("p (h e) -> p h e", h=H),
        )
        nc.vector.memset(vt[:, :, HD:HD + 1], 1.0)
        vo[st] = vt

    def emit_wpair(hp):
        """DMA + cast of this pair's W_q / W_k column slices (no PE work)."""
        wpair = []
        for dc in range(DC):
            wst = stage.tile([128, 256], F32, tag="wpst", bufs=4, name="wpst")
            nc.sync.dma_start(
                out=wst[:, 0:128],
                in_=wqkv_d.ap()[128 * dc:128 * (dc + 1), 128 * hp:128 * (hp + 1)],
            )
            nc.sync.dma_start(
                out=wst[:, 128:256],
                in_=wqkv_d.ap()[128 * dc:128 * (dc + 1),
                                D + 128 * hp:D + 128 * (hp + 1)],
            )
            wbf = wpairp.tile([128, 256], BF16, tag=f"wqk{dc}", bufs=2,
                              name=f"wqk{dc}")
            nc.vector.tensor_copy(wbf, wst)
            wpair.append(wbf)
        return wpair

    def qk_pieces(hp, wpair, sink):
        """Return a list of closures, each emitting one PE matmul (plus the
        occasional DVE bias-copy) of pair hp's Q^T/K^T projections. Executing
        all of them fills sink dict with qt/kt tiles."""
        qt = qktp.tile([128, SH], BF16, tag="qt", bufs=2, name=f"qt{hp}")
        kt = qktp.tile([128, S], BF16, tag="kt", bufs=2, name=f"kt{hp}")
        sink["qt"], sink["kt"] = qt, kt
        pieces = []
        holder = {}

        def q_mm(dc, nb):
            def f():
                if "qps" not in holder:
                    holder["qps"] = big_psum(f"qps{hp}")
                nc.tensor.matmul(
                    holder["qps"][:, 512 * nb:512 * (nb + 1)],
                    wpair[dc][:, 0:128],
                    xt_bf[dc][:, 512 * nb:512 * (nb + 1)],
                    start=(dc == 0),
                    stop=(dc == DC - 1),
                )
                if dc == DC - 1 and nb == QB - 1:
                    nc.vector.tensor_scalar_add(qt, holder["qps"],
                                                bq_cols[:, hp:hp + 1])
            return f

        def k_mm(kb, dc, nb):
            def f():
                key = f"kps{kb}"
                if key not in holder:
                    holder[key] = big_psum(f"kps{hp}_{kb}")
                nc.tensor.matmul(
                    holder[key][:, 512 * nb:512 * (nb + 1)],
                    wpair[dc][:, 128:256],
                    xt_bf[dc][:, SH * kb + 512 * nb:SH * kb + 512 * (nb + 1)],
                    start=(dc == 0),
                    stop=(dc == DC - 1),
                )
                if dc == DC - 1 and nb == 1:
                    nc.vector.tensor_scalar_add(
                        kt[:, SH * kb:SH * (kb + 1)], holder[key],
                        bq_cols[:, 8 + hp:9 + hp])
            return f

        for dc in range(DC):
            for nb in range(QB):
                pieces.append(q_mm(dc, nb))
        for kb in range(2):
            for dc in range(DC):
                for nb in range(2):
                    pieces.append(k_mm(kb, dc, nb))
        return pieces

    pending_norm = [None]

    def flush_norm():
        if pending_norm[0] is not None:
            pending_norm[0]()
            pending_norm[0] = None

    def emit_norm(hp, qb, pv, ott):
        """Normalize one query block: sums row -> partition-broadcast via a
        tiny f32r matmul (ones column) -> reciprocal -> scale output rows.
        Deferred into the NEXT block's first step so this chain (DVE+PE+DVE)
        never sits between two exp bursts on the critical path."""
        bc = big_psum(f"bc{hp}_{qb}")
        for h in range(2):
            sums = recp.tile([HD + 1, 512], F32R, tag=f"sums{h}", bufs=2,
                             name=f"sums{h}")
            nc.vector.tensor_copy(sums[HD:HD + 1, :], pv[h][HD:HD + 1, :])
            nc.tensor.matmul(
                bc[0:HD, 512 * h:512 * (h + 1)],
                ones_bc[HD:HD + 1, :],
                sums[HD:HD + 1, :],
                start=True,
                stop=True,
                tile_position=(64, 0),
            )
        for h in range(2):
            recb = recp.tile([64, 512], F32, tag=f"recb{h}", bufs=2,
                             name=f"recb{h}")
            nc.vector.reciprocal_approx_fast(
                recb, bc[0:HD, 512 * h:512 * (h + 1)])
            if h == 0:
                nc.vector.tensor_mul(
                    ott[0:64, 512 * qb:512 * (qb + 1)], pv[h][0:HD, :], recb
                )
            else:
                # head1 rows live at partitions 64:128 of ott; DVE is
                # partition-locked, so compute at base 0 and DMA-shift.
                otmp = recp.tile([64, 512], BF16, tag="otmp", bufs=2,
                                 name="otmp")
                nc.vector.tensor_mul(otmp, pv[h][0:HD, :], recb)
                nc.sync.dma_start(
                    out=ott[64:128, 512 * qb:512 * (qb + 1)], in_=otmp
                )

    def attention_pair(hp, qt, kt, fillers):
        """Attention for head pair hp. fillers: per-step closures emitting
        unrelated PE work (V tiles / next pair's QK matmuls) so the PE
        stream stays dense while ACT paces the exp chain."""
        nfill = len(fillers)
        fi = 0
        ott = otp.tile([128, SH], BF16, tag=f"ot{hp}", name=f"ot{hp}")
        for qb in range(QB):
            pv = [
                psum.tile([128, 512], F32, tag=f"pv{h}", bufs=1, name=f"pv{h}")
                for h in range(2)
            ]
            for kc in range(KC):
                step = qb * KC + kc
                # V for this key chunk must exist before its PV matmul
                if hp == 0 and qb == 0:
                    emit_v(kc)
                if hp == 0:
                    # qb0 already carries the V emission; fillers go in qb1
                    want = 0 if qb == 0 else (kc + 1) * nfill // KC
                else:
                    want = (step + 1) * nfill // (2 * KC)
                while fi < want:
                    fillers[fi]()
                    fi += 1
                scps = big_psum(f"sc{hp}_{qb}_{kc}")
                for h in range(2):
                    nc.tensor.matmul(
                        scps[:, 512 * h:512 * (h + 1)],
                        kt[64 * h:64 * (h + 1), 128 * kc:128 * (kc + 1)],
                        qt[64 * h:64 * (h + 1), 512 * qb:512 * (qb + 1)],
                        start=True,
                        stop=True,
                    )
                pt = ptp.tile([128, 1024], BF16, tag="pt", bufs=8, name="pt")
                nc.scalar.activation(pt, scps, mybir.ActivationFunctionType.Exp,
                                     scale=SCALE)
                if kc == 0:
                    flush_norm()
                for h in range(2):
                    nc.tensor.matmul(
                        pv[h][0:HD + 1, :],
                        vo[kc][:, 2 * hp + h, :],
                        pt[:, 512 * h:512 * (h + 1)],
                        start=(kc == 0),
                        stop=(kc == KC - 1),
                    )
            pending_norm[0] = (
                lambda hp=hp, qb=qb, pv=pv, ott=ott: emit_norm(hp, qb, pv, ott)
            )
        return ott

    # ---- pair pipeline ------------------------------------------------
    wpair0 = emit_wpair(0)
    sink = {}
    for piece in qk_pieces(0, wpair0, sink):
        piece()

    wproj_bf = []
    ot = []
    for hp in range(PAIRS):
        qt, kt = sink["qt"], sink["kt"]
        fillers = []
        if hp < PAIRS - 1:
            wpair_n = emit_wpair(hp + 1)
            sink = {}
            fillers = qk_pieces(hp + 1, wpair_n, sink)
        if hp == 0:
            # W_proj loads ride in the wv slots, which free after the last
            # V tile; emit now so the DMA engine fills them mid-attention.
            for dc in range(DC):
                wst = stage.tile([128, D], F32, tag="wst", bufs=2, name="wst")
                nc.sync.dma_start(
                    out=wst, in_=wproj_d.ap()[128 * dc:128 * (dc + 1), :])
                wbf = wvp.tile([128, D], BF16, tag=f"wv{dc}", bufs=1,
                               name=f"wvp{dc}")
                nc.vector.tensor_copy(wbf, wst)
                wproj_bf.append(wbf)
        ot.append(attention_pair(hp, qt, kt, fillers))

    # ---- output projection -------------------------------------------
    flush_norm()
    for qi in range(SH // 128):
        pps = big_psum(f"pps{qi}")
        for hp in range(PAIRS):
            for nb in range(2):
                nc.tensor.matmul(
                    pps[:, 512 * nb:512 * (nb + 1)],
                    ot[hp][:, 128 * qi:128 * (qi + 1)],
                    wproj_bf[hp][:, 512 * nb:512 * (nb + 1)],
                    start=(hp == 0),
                    stop=(hp == PAIRS - 1),
                )
        ost = outp.tile([128, D], F32, tag="ost", bufs=2, name="ost")
        nc.vector.tensor_add(ost, pps, bp_bcast)
        nc.sync.dma_start(out=out_d.ap()[128 * qi:128 * (qi + 1), :], in_=ost)


def build_nc():
    from contextlib import ExitStack

    nc = bacc.Bacc("TRN2", target_bir_lowering=False, debug=False,
                   num_devices=NCORES)
    with tile.TileContext(nc) as tc:
        with ExitStack() as ctx:
            build_graph(nc, tc, ctx)
    nc.compile()
    return nc


def make_in_maps(X, W_qkv, b_qkv, W_proj, b_proj):
    X = np.asarray(X, dtype=np.float32)
    wqkv = np.ascontiguousarray(np.asarray(W_qkv, dtype=np.float32))
    bqkv = np.ascontiguousarray(np.asarray(b_qkv, dtype=np.float32))
    wproj = np.ascontiguousarray(np.asarray(W_proj, dtype=np.float32))
    bproj = np.ascontiguousarray(np.asarray(b_proj, dtype=np.float32))
    xts = [np.ascontiguousarray(X[b].T) for b in range(B)]
    in_maps = []
    for i in range(NCORES):
        b, h = divmod(i, 2)
        xt = xts[b] if h == 0 else np.ascontiguousarray(
            np.roll(xts[b], -SH, axis=1)
        )
        in_maps.append({
            "XT": xt, "Wqkv": wqkv, "bqkv": bqkv,
            "Wproj": wproj, "bproj": bproj,
        })
    return in_maps


_NC_CACHE = None


def get_nc():
    global _NC_CACHE
    if _NC_CACHE is None:
        _NC_CACHE = build_nc()
    return _NC_CACHE


def kernel(X, W_qkv, b_qkv, W_proj, b_proj):
    nc = get_nc()
    in_maps = make_in_maps(X, W_qkv, b_qkv, W_proj, b_proj)
    res = run_bass_kernel_spmd(nc, in_maps, core_ids=list(range(NCORES)))
    out = np.empty((B, S, D), np.float32)
    for i in range(NCORES):
        b, h = divmod(i, 2)
        out[b, h * SH:(h + 1) * SH] = res.results[i]["out"]
    return out


# revision 16
# speedup vs baseline: 1.2405x; 1.0214x over previous
"""Multi-head attention block on 8 TRN2 NeuronCores.

Sharding: data-parallel over (batch b, query-half h) -> core i = 2*b + h.
Each core computes Q for its 1024 query tokens and K/V for the full 2048-token
sequence of its batch locally (duplicated across the pair), so no collectives
are needed.

Per-core layout choices:
- X is passed host-transposed as XT = X[b].T [D, S] f32, rolled so the core's
  own query tokens occupy columns 0:1023 (attention is permutation-invariant
  over key positions, so the roll is harmless for K/V).
- All matmuls run in bf16 (1 cyc/row on PE vs 4 for fp32), accumulating f32
  in PSUM.
- Scores are computed transposed (scoresT[k, q] = K @ Q^T) so softmax exp is a
  single ACT pass per key-chunk and the denominator falls out of the PV matmul
  via a ones column appended to V ([v_h | 1] lhsT -> psum row 64 = sum_k exp).
- The V projection and the next pair's Q/K projections are emitted interleaved
  into the attention key-chunk loop, so the (in-order) PE stream has no
  head-of-line stalls waiting on the exp stream and the ACT engine never
  starves at pair boundaries.
"""

import numpy as np

import concourse.bass as bass
import concourse.tile as tile
from concourse import bacc, mybir
from concourse.bass_utils import run_bass_kernel_spmd

F32 = mybir.dt.float32
F32R = mybir.dt.float32r
BF16 = mybir.dt.bfloat16

B, S, D = 4, 2048, 1024
H, HD = 16, 64
SH = S // 2          # query tokens per core
NCORES = 8
PAIRS = H // 2       # head pairs, processed 2 heads at a time
DC = D // 128        # contraction chunks of 128
ST = S // 128        # key-token tiles of 128
QB = SH // 512       # query blocks of 512
KC = S // 128        # key chunks of 128
SCALE = 1.0 / np.sqrt(HD)


def _pbcast1d(ap1d, parts):
    """[N] AP -> [parts, N] AP with partition stride 0 (DMA broadcast)."""
    return bass.AP(tensor=ap1d.tensor, offset=ap1d.offset,
                   ap=[[0, parts]] + list(ap1d.ap))


def build_graph(nc, tc, ctx):
    xt_d = nc.dram_tensor("XT", [D, S], F32, kind="ExternalInput")
    wqkv_d = nc.dram_tensor("Wqkv", [D, 3 * D], F32, kind="ExternalInput")
    bqkv_d = nc.dram_tensor("bqkv", [3 * D], F32, kind="ExternalInput")
    wproj_d = nc.dram_tensor("Wproj", [D, D], F32, kind="ExternalInput")
    bproj_d = nc.dram_tensor("bproj", [D], F32, kind="ExternalInput")
    out_d = nc.dram_tensor("out", [SH, D], F32, kind="ExternalOutput")

    const = ctx.enter_context(tc.tile_pool(name="const", bufs=1))
    stage = ctx.enter_context(tc.tile_pool(name="stage", bufs=2))
    xtp = ctx.enter_context(tc.tile_pool(name="xtp", bufs=1))
    wvp = ctx.enter_context(tc.tile_pool(name="wvp", bufs=1))
    wpairp = ctx.enter_context(tc.tile_pool(name="wpairp", bufs=2))
    qktp = ctx.enter_context(tc.tile_pool(name="qktp", bufs=2))
    vop = ctx.enter_context(tc.tile_pool(name="vop", bufs=1))
    ptp = ctx.enter_context(tc.tile_pool(name="ptp", bufs=8))
    otp = ctx.enter_context(tc.tile_pool(name="otp", bufs=1))
    recp = ctx.enter_context(tc.tile_pool(name="recp", bufs=2))
    outp = ctx.enter_context(tc.tile_pool(name="outp", bufs=2))
    psum = ctx.enter_context(tc.tile_pool(name="psum", bufs=1, space="PSUM"))

    def big_psum(name):
        return psum.tile([128, 1024], F32, tag="big", bufs=3, name=name)

    # ---- pair-0 W_q/W_k slices first: QK(0) gates the first exp ------
    wpair0_st = []
    for dc in range(DC):
        wst = stage.tile([128, 256], F32, tag="wpst", bufs=4, name="wpst")
        nc.sync.dma_start(
            out=wst[:, 0:128],
            in_=wqkv_d.ap()[128 * dc:128 * (dc + 1), 0:128],
        )
        nc.sync.dma_start(
            out=wst[:, 128:256],
            in_=wqkv_d.ap()[128 * dc:128 * (dc + 1), D:D + 128],
        )
        wpair0_st.append(wst)

    # ---- X^T load + cast to bf16, W_v interleaved --------------------
    xt_st = []
    for dc in range(DC):
        xst = stage.tile([128, S], F32, tag="xst", bufs=3, name="xst")
        nc.sync.dma_start(out=xst, in_=xt_d.ap()[128 * dc:128 * (dc + 1), :])
        xt_st.append(xst)
    wv_bf = []
    for dc in range(DC):
        wst = stage.tile([128, D], F32, tag="wst", bufs=2, name="wst")
        nc.sync.dma_start(out=wst,
                          in_=wqkv_d.ap()[128 * dc:128 * (dc + 1), 2 * D:3 * D])
        wbf = wvp.tile([128, D], BF16, tag=f"wv{dc}", bufs=1, name=f"wv{dc}")
        nc.vector.tensor_copy(wbf, wst)
        wv_bf.append(wbf)
    xt_bf = []
    for dc in range(DC):
        xbf = xtp.tile([128, S], BF16, tag=f"xt{dc}", name=f"xt{dc}")
        nc.vector.tensor_copy(xbf, xt_st[dc])
        xt_bf.append(xbf)

    # ---- biases (gpsimd queue so the gather doesn't block big loads) --
    bq_cols = const.tile([128, 24], F32)
    nc.gpsimd.dma_start(out=bq_cols,
                        in_=bqkv_d.ap().rearrange("(c p) -> p c", p=128))
    bv_bcast = const.tile([128, D], F32)
    nc.gpsimd.dma_start(out=bv_bcast, in_=_pbcast1d(bqkv_d.ap()[2 * D:3 * D], 128))
    bp_bcast = const.tile([128, D], F32)
    nc.gpsimd.dma_start(out=bp_bcast, in_=_pbcast1d(bproj_d.ap(), 128))
    ones_f = const.tile([HD + 1, HD], F32)
    nc.vector.memset(ones_f, 1.0)
    ones_bc = const.tile([HD + 1, HD], F32R)
    nc.vector.tensor_copy(ones_bc, ones_f)

    # ---- deferred-emission pieces ------------------------------------
    vo = [None] * ST

    def emit_v(st):
        """V = X @ W_v + b_v for one 128-token tile, as [128, H, HD+1]."""
        vps = big_psum(f"vps{st}")
        for dc in range(DC):
            for nb in range(2):
                nc.tensor.matmul(
                    vps[:, 512 * nb:512 * (nb + 1)],
                    xt_bf[dc][:, 128 * st:128 * (st + 1)],
                    wv_bf[dc][:, 512 * nb:512 * (nb + 1)],
                    start=(dc == 0),
                    stop=(dc == DC - 1),
                )
        vt = vop.tile([128, H, HD + 1], BF16, tag=f"vo{st}", name=f"vo{st}")
        nc.vector.tensor_add(
            vt[:, :, 0:HD],
            vps.rearrange("p (h e) -> p h e", h=H),
            bv_bcast.rearrange("p (h e) -> p h e", h=H),
        )
        nc.vector.memset(vt[:, :, HD:HD + 1], 1.0)
        vo[st] = vt

    def emit_wpair(hp, preloaded=None):
        """DMA + cast of this pair's W_q / W_k column slices (no PE work)."""
        wpair = []
        for dc in range(DC):
            if preloaded is not None:
                wst = preloaded[dc]
            else:
                wst = stage.tile([128, 256], F32, tag="wpst", bufs=4,
                                 name="wpst")
                nc.sync.dma_start(
                    out=wst[:, 0:128],
                    in_=wqkv_d.ap()[128 * dc:128 * (dc + 1),
                                    128 * hp:128 * (hp + 1)],
                )
                nc.sync.dma_start(
                    out=wst[:, 128:256],
                    in_=wqkv_d.ap()[128 * dc:128 * (dc + 1),
                                    D + 128 * hp:D + 128 * (hp + 1)],
                )
            wbf = wpairp.tile([128, 256], BF16, tag=f"wqk{dc}", bufs=2,
                              name=f"wqk{dc}")
            nc.vector.tensor_copy(wbf, wst)
            wpair.append(wbf)
        return wpair

    def qk_pieces(hp, wpair, sink):
        """Return a list of closures, each emitting one PE matmul (plus the
        occasional DVE bias-copy) of pair hp's Q^T/K^T projections. Executing
        all of them fills sink dict with qt/kt tiles."""
        qt = qktp.tile([128, SH], BF16, tag="qt", bufs=2, name=f"qt{hp}")
        kt = qktp.tile([128, S], BF16, tag="kt", bufs=2, name=f"kt{hp}")
        sink["qt"], sink["kt"] = qt, kt
        pieces = []
        holder = {}

        def q_mm(dc, nb):
            def f():
                if "qps" not in holder:
                    holder["qps"] = big_psum(f"qps{hp}")
                nc.tensor.matmul(
                    holder["qps"][:, 512 * nb:512 * (nb + 1)],
                    wpair[dc][:, 0:128],
                    xt_bf[dc][:, 512 * nb:512 * (nb + 1)],
                    start=(dc == 0),
                    stop=(dc == DC - 1),
                )
                if dc == DC - 1 and nb == QB - 1:
                    nc.vector.tensor_scalar_add(qt, holder["qps"],
                                                bq_cols[:, hp:hp + 1])
            return f

        def k_mm(kb, dc, nb):
            def f():
                key = f"kps{kb}"
                if key not in holder:
                    holder[key] = big_psum(f"kps{hp}_{kb}")
                nc.tensor.matmul(
                    holder[key][:, 512 * nb:512 * (nb + 1)],
                    wpair[dc][:, 128:256],
                    xt_bf[dc][:, SH * kb + 512 * nb:SH * kb + 512 * (nb + 1)],
                    start=(dc == 0),
                    stop=(dc == DC - 1),
                )
                if dc == DC - 1 and nb == 1:
                    nc.vector.tensor_scalar_add(
                        kt[:, SH * kb:SH * (kb + 1)], holder[key],
                        bq_cols[:, 8 + hp:9 + hp])
            return f

        for dc in range(DC):
            for nb in range(QB):
                pieces.append(q_mm(dc, nb))
        for kb in range(2):
            for dc in range(DC):
                for nb in range(2):
                    pieces.append(k_mm(kb, dc, nb))
        return pieces

    pending_pv = [None]

    def emit_pv(hp, pv, kc, pt):
        for h in range(2):
            nc.tensor.matmul(
                pv[h][0:HD + 1, :],
                vo[kc][:, 2 * hp + h, :],
                pt[:, 512 * h:512 * (h + 1)],
                start=(kc == 0),
                stop=(kc == KC - 1),
            )

    pending_norm = [None]

    def flush_norm():
        if pending_norm[0] is not None:
            pending_norm[0]()
            pending_norm[0] = None

    def emit_norm(hp, qb, pv, ott):
        """Normalize one query block: sums row -> partition-broadcast via a
        tiny f32r matmul (ones column) -> reciprocal -> scale output rows.
        Deferred into the NEXT block's first step so this chain (DVE+PE+DVE)
        never sits between two exp bursts on the critical path."""
        bc = big_psum(f"bc{hp}_{qb}")
        for h in range(2):
            sums = recp.tile([HD + 1, 512], F32R, tag=f"sums{h}", bufs=2,
                             name=f"sums{h}")
            nc.vector.tensor_copy(sums[HD:HD + 1, :], pv[h][HD:HD + 1, :])
            nc.tensor.matmul(
                bc[0:HD, 512 * h:512 * (h + 1)],
                ones_bc[HD:HD + 1, :],
                sums[HD:HD + 1, :],
                start=True,
                stop=True,
                tile_position=(64, 0),
            )
        for h in range(2):
            recb = recp.tile([64, 512], F32, tag=f"recb{h}", bufs=2,
                             name=f"recb{h}")
            nc.vector.reciprocal_approx_fast(
                recb, bc[0:HD, 512 * h:512 * (h + 1)])
            if h == 0:
                nc.vector.tensor_mul(
                    ott[0:64, 512 * qb:512 * (qb + 1)], pv[h][0:HD, :], recb
                )
            else:
                # head1 rows live at partitions 64:128 of ott; DVE is
                # partition-locked, so compute at base 0 and DMA-shift.
                otmp = recp.tile([64, 512], BF16, tag="otmp", bufs=2,
                                 name="otmp")
                nc.vector.tensor_mul(otmp, pv[h][0:HD, :], recb)
                nc.sync.dma_start(
                    out=ott[64:128, 512 * qb:512 * (qb + 1)], in_=otmp
                )

    def attention_pair(hp, qt, kt, fillers):
        """Attention for head pair hp. fillers: per-step closures emitting
        unrelated PE work (V tiles / next pair's QK matmuls) so the PE
        stream stays dense while ACT paces the exp chain."""
        nfill = len(fillers)
        fi = 0
        ott = otp.tile([128, SH], BF16, tag=f"ot{hp}", name=f"ot{hp}")
        for qb in range(QB):
            pv = [
                psum.tile([128, 512], F32, tag=f"pv{h}", bufs=1, name=f"pv{h}")
                for h in range(2)
            ]
            for kc in range(KC):
                step = qb * KC + kc
                # V for this key chunk must exist before its PV matmul
                if hp == 0 and qb == 0:
                    emit_v(kc)
                if hp == 0:
                    # qb0 already carries the V emission; fillers go in qb1
                    want = 0 if qb == 0 else (kc + 1) * nfill // KC
                else:
                    want = (step + 1) * nfill // (2 * KC)
                while fi < want:
                    fillers[fi]()
                    fi += 1
                scps = big_psum(f"sc{hp}_{qb}_{kc}")
                for h in range(2):
                    nc.tensor.matmul(
                        scps[:, 512 * h:512 * (h + 1)],
                        kt[64 * h:64 * (h + 1), 128 * kc:128 * (kc + 1)],
                        qt[64 * h:64 * (h + 1), 512 * qb:512 * (qb + 1)],
                        start=True,
                        stop=True,
                    )
                pt = ptp.tile([128, 1024], BF16, tag="pt", bufs=8, name="pt")
                nc.scalar.activation(pt, scps, mybir.ActivationFunctionType.Exp,
                                     scale=SCALE)
                # PV runs one step behind sc/exp (and the previous block's
                # last PV + normalize drain here), so the in-order PE stream
                # never blocks the next exp on the current PV.
                if pending_pv[0] is not None:
                    pending_pv[0]()
                if kc == 0:
                    flush_norm()
                pending_pv[0] = (
                    lambda hp=hp, pv=pv, kc=kc, pt=pt: emit_pv(hp, pv, kc, pt)
                )
            pending_norm[0] = (
                lambda hp=hp, qb=qb, pv=pv, ott=ott: emit_norm(hp, qb, pv, ott)
            )
        return ott

    # ---- pair pipeline ------------------------------------------------
    wpair0 = emit_wpair(0, wpair0_st)
    sink = {}
    for piece in qk_pieces(0, wpair0, sink):
        piece()

    wproj_bf = []
    ot = []
    for hp in range(PAIRS):
        qt, kt = sink["qt"], sink["kt"]
        fillers = []
        if hp < PAIRS - 1:
            wpair_n = emit_wpair(hp + 1)
            sink = {}
            fillers = qk_pieces(hp + 1, wpair_n, sink)
        if hp == 0:
            # W_proj loads ride in the wv slots, which free after the last
            # V tile; emit now so the DMA engine fills them mid-attention.
            for dc in range(DC):
                wst = stage.tile([128, D], F32, tag="wst", bufs=2, name="wst")
                nc.sync.dma_start(
                    out=wst, in_=wproj_d.ap()[128 * dc:128 * (dc + 1), :])
                wbf = wvp.tile([128, D], BF16, tag=f"wv{dc}", bufs=1,
                               name=f"wvp{dc}")
                nc.vector.tensor_copy(wbf, wst)
                wproj_bf.append(wbf)
        ot.append(attention_pair(hp, qt, kt, fillers))

    # ---- output projection -------------------------------------------
    if pending_pv[0] is not None:
        pending_pv[0]()
        pending_pv[0] = None
    flush_norm()
    for qi in range(SH // 128):
        pps = big_psum(f"pps{qi}")
        for hp in range(PAIRS):
            for nb in range(2):
                nc.tensor.matmul(
                    pps[:, 512 * nb:512 * (nb + 1)],
                    ot[hp][:, 128 * qi:128 * (qi + 1)],
                    wproj_bf[hp][:, 512 * nb:512 * (nb + 1)],
                    start=(hp == 0),
                    stop=(hp == PAIRS - 1),
                )
        ost = outp.tile([128, D], F32, tag="ost", bufs=2, name="ost")
        nc.vector.tensor_add(ost, pps, bp_bcast)
        nc.sync.dma_start(out=out_d.ap()[128 * qi:128 * (qi + 1), :], in_=ost)


def build_nc():
    from contextlib import ExitStack

    nc = bacc.Bacc("TRN2", target_bir_lowering=False, debug=False,
                   num_devices=NCORES)
    with tile.TileContext(nc) as tc:
        with ExitStack() as ctx:
            build_graph(nc, tc, ctx)
    nc.compile()
    return nc


def make_in_maps(X, W_qkv, b_qkv, W_proj, b_proj):
    X = np.asarray(X, dtype=np.float32)
    wqkv = np.ascontiguousarray(np.asarray(W_qkv, dtype=np.float32))
    bqkv = np.ascontiguousarray(np.asarray(b_qkv, dtype=np.float32))
    wproj = np.ascontiguousarray(np.asarray(W_proj, dtype=np.float32))
    bproj = np.ascontiguousarray(np.asarray(b_proj, dtype=np.float32))
    xts = [np.ascontiguousarray(X[b].T) for b in range(B)]
    in_maps = []
    for i in range(NCORES):
        b, h = divmod(i, 2)
        xt = xts[b] if h == 0 else np.ascontiguousarray(
            np.roll(xts[b], -SH, axis=1)
        )
        in_maps.append({
            "XT": xt, "Wqkv": wqkv, "bqkv": bqkv,
            "Wproj": wproj, "bproj": bproj,
        })
    return in_maps


_NC_CACHE = None


def get_nc():
    global _NC_CACHE
    if _NC_CACHE is None:
        _NC_CACHE = build_nc()
    return _NC_CACHE


def kernel(X, W_qkv, b_qkv, W_proj, b_proj):
    nc = get_nc()
    in_maps = make_in_maps(X, W_qkv, b_qkv, W_proj, b_proj)
    res = run_bass_kernel_spmd(nc, in_maps, core_ids=list(range(NCORES)))
    out = np.empty((B, S, D), np.float32)
    for i in range(NCORES):
        b, h = divmod(i, 2)
        out[b, h * SH:(h + 1) * SH] = res.results[i]["out"]
    return out


# revision 19
# speedup vs baseline: 1.3024x; 1.0499x over previous
"""Multi-head attention block on 8 TRN2 NeuronCores.

Sharding v6: tensor-parallel over heads within each batch pair.
Core i = 2*b + hh handles batch b and HEAD half hh (8 of 16 heads), computing
Q/K/V for its heads over the FULL 2048-token sequence. This halves the K and
V projection work versus duplicating them per query-half. After attention,
the cores of a pair exchange the half of the (transposed) attention output
each needs via a 256KB pairwise AllGather per head-pair group, then each core
runs the full output projection for its 1024 query tokens.

SPMD trick 1 (query order): XT is passed host-transposed and column-rolled so
each core's OWN query tokens occupy columns 0:1023 (attention is permutation-
invariant over key positions, so the roll is harmless for K/V). All graph
slices are then per-core-constant.
SPMD trick 2 (exchange): the AllGather output chunk a core needs is indexed
by its partner's rank, which would be a per-core constant. Instead both
chunks are summed and the core's own contribution subtracted (bf16 a+b-a in
f32 intermediate is bit-exact), which is rank-agnostic.

Layout choices (unchanged from earlier versions):
- bf16 matmuls (1 cyc/row on PE), f32 PSUM accumulation.
- Scores computed transposed (scoresT[k, q] = K @ Q^T) so softmax exp is one
  ACT pass per key chunk and the denominator falls out of the PV matmul via a
  ones column appended to V.
- V and the next pair's Q/K projections are interleaved into the attention
  key-chunk loop; PV runs one step behind sc/exp; per-block normalize and the
  exchange are deferred into the next block's first step. This keeps the
  in-order PE stream free of head-of-line stalls against the ACT exp chain.
"""

import numpy as np

import concourse.bass as bass
import concourse.tile as tile
from concourse import bacc, mybir
from concourse.bass_utils import run_bass_kernel_spmd

F32 = mybir.dt.float32
F32R = mybir.dt.float32r
BF16 = mybir.dt.bfloat16

B, S, D = 4, 2048, 1024
H, HD = 16, 64
SH = S // 2          # query tokens written out per core
NCORES = 8
LPAIRS = 4           # local head pairs per core (8 heads)
DC = D // 128        # contraction chunks of 128
ST = S // 128        # key-token tiles of 128
QB = S // 512        # query blocks of 512 (full sequence)
KC = S // 128        # key chunks of 128
SCALE = 1.0 / np.sqrt(HD)
GROUPS = [[0, 1], [2, 3], [4, 5], [6, 7]]


def _pbcast1d(ap1d, parts):
    """[N] AP -> [parts, N] AP with partition stride 0 (DMA broadcast)."""
    return bass.AP(tensor=ap1d.tensor, offset=ap1d.offset,
                   ap=[[0, parts]] + list(ap1d.ap))


def build_graph(nc, tc, ctx):
    # Wqkv: host-sliced per core: [D, 1536] = [Q cols | K cols | V cols] for
    # this core's 8 heads. Wproj: host-reordered rows: [my-head rows (512) |
    # partner-head rows (512)]. bqkT: [128, 8] per-row-block bias columns
    # (Q pairs 0-3, K pairs 4-7). bv: [512] V bias for my heads.
    xt_d = nc.dram_tensor("XT", [D, S], F32, kind="ExternalInput")
    wqkv_d = nc.dram_tensor("Wqkv", [D, 1536], F32, kind="ExternalInput")
    bqkt_d = nc.dram_tensor("bqkT", [128, 8], F32, kind="ExternalInput")
    bv_d = nc.dram_tensor("bv", [512], F32, kind="ExternalInput")
    wproj_d = nc.dram_tensor("Wproj", [D, D], F32, kind="ExternalInput")
    bproj_d = nc.dram_tensor("bproj", [D], F32, kind="ExternalInput")
    out_d = nc.dram_tensor("out", [SH, D], F32, kind="ExternalOutput")

    const = ctx.enter_context(tc.tile_pool(name="const", bufs=1))
    stage = ctx.enter_context(tc.tile_pool(name="stage", bufs=2))
    xtp = ctx.enter_context(tc.tile_pool(name="xtp", bufs=1))
    wvp = ctx.enter_context(tc.tile_pool(name="wvp", bufs=1))
    wpp = ctx.enter_context(tc.tile_pool(name="wpp", bufs=1))
    wpairp = ctx.enter_context(tc.tile_pool(name="wpairp", bufs=2))
    qktp = ctx.enter_context(tc.tile_pool(name="qktp", bufs=2))
    vop = ctx.enter_context(tc.tile_pool(name="vop", bufs=1))
    ptp = ctx.enter_context(tc.tile_pool(name="ptp", bufs=8))
    otp = ctx.enter_context(tc.tile_pool(name="otp", bufs=1))
    rotp = ctx.enter_context(tc.tile_pool(name="rotp", bufs=1))
    recp = ctx.enter_context(tc.tile_pool(name="recp", bufs=2))
    outp = ctx.enter_context(tc.tile_pool(name="outp", bufs=2))
    dramp = ctx.enter_context(tc.tile_pool(name="dramp", bufs=1, space="DRAM"))
    psum = ctx.enter_context(tc.tile_pool(name="psum", bufs=1, space="PSUM"))

    def big_psum(name):
        return psum.tile([128, 1024], F32, tag="big", bufs=3, name=name)

    # ---- pair-0 W slices first: QK(0) gates the first exp ------------
    wpair0_st = []
    for dc in range(DC):
        wst = stage.tile([128, 256], F32, tag="wpst", bufs=4, name="wpst")
        nc.sync.dma_start(out=wst[:, 0:128],
                          in_=wqkv_d.ap()[128 * dc:128 * (dc + 1), 0:128])
        nc.sync.dma_start(out=wst[:, 128:256],
                          in_=wqkv_d.ap()[128 * dc:128 * (dc + 1), 512:640])
        wpair0_st.append(wst)

    # ---- X^T load + cast to bf16, W_v interleaved --------------------
    xt_st = []
    for dc in range(DC):
        xst = stage.tile([128, S], F32, tag="xst", bufs=2, name="xst")
        nc.sync.dma_start(out=xst, in_=xt_d.ap()[128 * dc:128 * (dc + 1), :])
        xt_st.append(xst)
    wv_bf = []
    for dc in range(DC):
        wst = stage.tile([128, 512], F32, tag="wst", bufs=2, name="wst")
        nc.sync.dma_start(out=wst,
                          in_=wqkv_d.ap()[128 * dc:128 * (dc + 1), 1024:1536])
        wbf = wvp.tile([128, 512], BF16, tag=f"wv{dc}", bufs=1, name=f"wv{dc}")
        nc.vector.tensor_copy(wbf, wst)
        wv_bf.append(wbf)
    xt_bf = []
    for dc in range(DC):
        xbf = xtp.tile([128, S], BF16, tag=f"xt{dc}", name=f"xt{dc}")
        nc.vector.tensor_copy(xbf, xt_st[dc])
        xt_bf.append(xbf)

    # ---- biases (gpsimd queue so they don't block the big loads) -----
    bq_cols = const.tile([128, 8], F32)
    nc.gpsimd.dma_start(out=bq_cols, in_=bqkt_d.ap())
    bv_bcast = const.tile([128, 512], F32)
    nc.gpsimd.dma_start(out=bv_bcast, in_=_pbcast1d(bv_d.ap(), 128))
    bp_bcast = const.tile([128, D], F32)
    nc.gpsimd.dma_start(out=bp_bcast, in_=_pbcast1d(bproj_d.ap(), 128))
    ones_f = const.tile([HD + 1, HD], F32)
    nc.vector.memset(ones_f, 1.0)
    ones_bc = const.tile([HD + 1, HD], F32R)
    nc.vector.tensor_copy(ones_bc, ones_f)

    # ---- deferred-emission pieces ------------------------------------
    vo = [None] * ST

    def emit_v(st):
        """V = X @ W_v + b_v for one 128-token tile, as [128, 8, HD+1]."""
        vps = big_psum(f"vps{st}")
        for dc in range(DC):
            nc.tensor.matmul(
                vps[:, 0:512],
                xt_bf[dc][:, 128 * st:128 * (st + 1)],
                wv_bf[dc],
                start=(dc == 0),
                stop=(dc == DC - 1),
            )
        vt = vop.tile([128, 8, HD + 1], BF16, tag=f"vo{st}", name=f"vo{st}")
        nc.vector.tensor_add(
            vt[:, :, 0:HD],
            vps[:, 0:512].rearrange("p (h e) -> p h e", h=8),
            bv_bcast.rearrange("p (h e) -> p h e", h=8),
        )
        nc.vector.memset(vt[:, :, HD:HD + 1], 1.0)
        vo[st] = vt

    def emit_wpair(hp, preloaded=None):
        """DMA + cast of this pair's W_q / W_k column slices (no PE work)."""
        wpair = []
        for dc in range(DC):
            if preloaded is not None:
                wst = preloaded[dc]
            else:
                wst = stage.tile([128, 256], F32, tag="wpst", bufs=4,
                                 name="wpst")
                nc.sync.dma_start(
                    out=wst[:, 0:128],
                    in_=wqkv_d.ap()[128 * dc:128 * (dc + 1),
                                    128 * hp:128 * (hp + 1)],
                )
                nc.sync.dma_start(
                    out=wst[:, 128:256],
                    in_=wqkv_d.ap()[128 * dc:128 * (dc + 1),
                                    512 + 128 * hp:512 + 128 * (hp + 1)],
                )
            wbf = wpairp.tile([128, 256], BF16, tag=f"wqk{dc}", bufs=2,
                              name=f"wqk{dc}")
            nc.vector.tensor_copy(wbf, wst)
            wpair.append(wbf)
        return wpair

    def qk_pieces(hp, wpair, sink):
        """Closures each emitting one PE matmul (plus occasional DVE bias
        copy) of pair hp's Q^T/K^T projections over the full sequence."""
        qt = qktp.tile([128, S], BF16, tag="qt", bufs=2, name=f"qt{hp}")
        kt = qktp.tile([128, S], BF16, tag="kt", bufs=2, name=f"kt{hp}")
        sink["qt"], sink["kt"] = qt, kt
        pieces = []
        holder = {}

        def qk_mm(which, half, dc, nb):
            wslice = (slice(0, 128) if which == "q" else slice(128, 256))
            bias_col = hp if which == "q" else 4 + hp
            dst = qt if which == "q" else kt

            def f():
                key = f"{which}{half}"
                if key not in holder:
                    holder[key] = big_psum(f"{which}ps{hp}_{half}")
                nc.tensor.matmul(
                    holder[key][:, 512 * nb:512 * (nb + 1)],
                    wpair[dc][:, wslice],
                    xt_bf[dc][:, SH * half + 512 * nb:SH * half + 512 * (nb + 1)],
                    start=(dc == 0),
                    stop=(dc == DC - 1),
                )
                if dc == DC - 1 and nb == 1:
                    nc.vector.tensor_scalar_add(
                        dst[:, SH * half:SH * (half + 1)], holder[key],
                        bq_cols[:, bias_col:bias_col + 1])
            return f

        for which in ("q", "k"):
            for half in range(2):
                for dc in range(DC):
                    for nb in range(2):
                        pieces.append(qk_mm(which, half, dc, nb))
        return pieces

    pending_pv = [None]

    def emit_pv(hp, pv, kc, pt):
        for h in range(2):
            nc.tensor.matmul(
                pv[h][0:HD + 1, :],
                vo[kc][:, 2 * hp + h, :],
                pt[:, 512 * h:512 * (h + 1)],
                start=(kc == 0),
                stop=(kc == KC - 1),
            )

    pending_norm = [None]

    def flush_norm():
        if pending_norm[0] is not None:
            pending_norm[0]()
            pending_norm[0] = None

    pending_xchg = [None]

    def flush_xchg():
        if pending_xchg[0] is not None:
            pending_xchg[0]()
            pending_xchg[0] = None

    def emit_norm(hp, qb, pv, ott):
        """Normalize one query block: sums row -> partition-broadcast via a
        tiny f32r matmul (ones column) -> reciprocal -> scale output rows."""
        bc = big_psum(f"bc{hp}_{qb}")
        for h in range(2):
            sums = recp.tile([HD + 1, 512], F32R, tag=f"sums{h}", bufs=2,
                             name=f"sums{h}")
            nc.vector.tensor_copy(sums[HD:HD + 1, :], pv[h][HD:HD + 1, :])
            nc.tensor.matmul(
                bc[0:HD, 512 * h:512 * (h + 1)],
                ones_bc[HD:HD + 1, :],
                sums[HD:HD + 1, :],
                start=True,
                stop=True,
                tile_position=(64, 0),
            )
        for h in range(2):
            recb = recp.tile([64, 512], F32, tag=f"recb{h}", bufs=2,
                             name=f"recb{h}")
            nc.vector.reciprocal_approx_fast(
                recb, bc[0:HD, 512 * h:512 * (h + 1)])
            if h == 0:
                nc.vector.tensor_mul(
                    ott[0:64, 512 * qb:512 * (qb + 1)], pv[h][0:HD, :], recb
                )
            else:
                # head1 rows live at partitions 64:128 of ott; DVE is
                # partition-locked, so compute at base 0 and DMA-shift.
                otmp = recp.tile([64, 512], BF16, tag="otmp", bufs=2,
                                 name="otmp")
                nc.vector.tensor_mul(otmp, pv[h][0:HD, :], recb)
                nc.sync.dma_start(
                    out=ott[64:128, 512 * qb:512 * (qb + 1)], in_=otmp
                )

    rot = [None] * LPAIRS

    def emit_xchg(hp, ott):
        """Pairwise exchange of the partner-half attention output rows via
        AllGather; recover the partner's chunk rank-agnostically as
        (chunk0 + chunk1) - own (bit-exact for bf16 in f32 intermediate)."""
        inb = dramp.tile([128, SH], BF16, name=f"inb{hp}")
        outb = dramp.tile([2, 128, SH], BF16, name=f"outb{hp}")
        nc.sync.dma_start(out=inb, in_=ott[:, SH:S])
        nc.gpsimd.collective_compute(
            "AllGather",
            mybir.AluOpType.bypass,
            ins=[inb.opt()],
            outs=[outb.opt()],
            replica_groups=GROUPS,
        )
        both = rotp.tile([128, 2, SH], BF16, tag="both", bufs=1,
                         name=f"both{hp}")
        nc.sync.dma_start(out=both, in_=outb.rearrange("c p n -> p c n"))
        ssum = rotp.tile([128, SH], F32, tag="ssum", bufs=1, name=f"ssum{hp}")
        nc.vector.tensor_add(ssum, both[:, 0, :], both[:, 1, :])
        rt = rotp.tile([128, SH], BF16, tag=f"rot{hp}", name=f"rot{hp}")
        nc.vector.tensor_sub(rt, ssum, ott[:, SH:S])
        rot[hp] = rt

    def attention_pair(hp, qt, kt, fillers):
        """Attention for local head pair hp over the full query sequence.
        fillers: per-step closures emitting unrelated PE work so the
        in-order PE stream stays dense while ACT paces the exp chain."""
        nfill = len(fillers)
        fi = 0
        ott = otp.tile([128, S], BF16, tag=f"ot{hp}", name=f"ot{hp}")
        for qb in range(QB):
            pv = [
                psum.tile([128, 512], F32, tag=f"pv{h}", bufs=1, name=f"pv{h}")
                for h in range(2)
            ]
            for kc in range(KC):
                step = qb * KC + kc
                # V for this key chunk must exist before its PV matmul
                if hp == 0 and qb == 0:
                    emit_v(kc)
                if hp == 0:
                    want = (0 if qb == 0
                            else (step - KC + 1) * nfill // (3 * KC))
                else:
                    want = (step + 1) * nfill // (QB * KC)
                while fi < want:
                    fillers[fi]()
                    fi += 1
                scps = big_psum(f"sc{hp}_{qb}_{kc}")
                for h in range(2):
                    nc.tensor.matmul(
                        scps[:, 512 * h:512 * (h + 1)],
                        kt[64 * h:64 * (h + 1), 128 * kc:128 * (kc + 1)],
                        qt[64 * h:64 * (h + 1), 512 * qb:512 * (qb + 1)],
                        start=True,
                        stop=True,
                    )
                pt = ptp.tile([128, 1024], BF16, tag="pt", bufs=8, name="pt")
                nc.scalar.activation(pt, scps, mybir.ActivationFunctionType.Exp,
                                     scale=SCALE)
                # PV runs one step behind sc/exp; previous block's last PV,
                # normalize and pair-exchange drain here.
                if pending_pv[0] is not None:
                    pending_pv[0]()
                if kc == 0:
                    flush_norm()
                    flush_xchg()
                pending_pv[0] = (
                    lambda hp=hp, pv=pv, kc=kc, pt=pt: emit_pv(hp, pv, kc, pt)
                )
            pending_norm[0] = (
                lambda hp=hp, qb=qb, pv=pv, ott=ott: emit_norm(hp, qb, pv, ott)
            )
        pending_xchg[0] = lambda hp=hp, ott=ott: emit_xchg(hp, ott)
        return ott

    # ---- pair pipeline ------------------------------------------------
    wpair0 = emit_wpair(0, wpair0_st)
    sink = {}
    for piece in qk_pieces(0, wpair0, sink):
        piece()

    wproj_bf = []
    ot = []
    for hp in range(LPAIRS):
        qt, kt = sink["qt"], sink["kt"]
        fillers = []
        if hp < LPAIRS - 1:
            wpair_n = emit_wpair(hp + 1)
            sink = {}
            fillers = qk_pieces(hp + 1, wpair_n, sink)
        if hp == 0:
            # W_proj loads fill mid-attention while the DMA queue is idle.
            for dc in range(DC):
                wst = stage.tile([128, D], F32, tag="wprs", bufs=1,
                                 name="wprs")
                nc.sync.dma_start(
                    out=wst, in_=wproj_d.ap()[128 * dc:128 * (dc + 1), :])
                wbf = wpp.tile([128, D], BF16, tag=f"wp{dc}", bufs=1,
                               name=f"wp{dc}")
                nc.vector.tensor_copy(wbf, wst)
                wproj_bf.append(wbf)
        ot.append(attention_pair(hp, qt, kt, fillers))

    # ---- drain deferred tail work ------------------------------------
    if pending_pv[0] is not None:
        pending_pv[0]()
        pending_pv[0] = None
    flush_norm()
    flush_xchg()

    # ---- output projection -------------------------------------------
    # contract rows: chunks 0-3 = local pairs (my heads), 4-7 = received
    # partner pairs (Wproj rows were host-reordered to match). The remote
    # chunks come last so the final exchange hides under the early chunks.
    for qi in range(SH // 128):
        pps = big_psum(f"pps{qi}")
        for c in range(8):
            lhs = (ot[c][:, 128 * qi:128 * (qi + 1)] if c < LPAIRS
                   else rot[c - LPAIRS][:, 128 * qi:128 * (qi + 1)])
            for nb in range(2):
                nc.tensor.matmul(
                    pps[:, 512 * nb:512 * (nb + 1)],
                    lhs,
                    wproj_bf[c][:, 512 * nb:512 * (nb + 1)],
                    start=(c == 0),
                    stop=(c == 7),
                )
        ost = outp.tile([128, D], F32, tag="ost", bufs=2, name="ost")
        nc.vector.tensor_add(ost, pps, bp_bcast)
        nc.sync.dma_start(out=out_d.ap()[128 * qi:128 * (qi + 1), :], in_=ost)


def build_nc():
    from contextlib import ExitStack

    nc = bacc.Bacc("TRN2", target_bir_lowering=False, debug=False,
                   num_devices=NCORES)
    with tile.TileContext(nc) as tc:
        with ExitStack() as ctx:
            build_graph(nc, tc, ctx)
    nc.compile()
    return nc


def make_in_maps(X, W_qkv, b_qkv, W_proj, b_proj):
    X = np.asarray(X, dtype=np.float32)
    wqkv = np.asarray(W_qkv, dtype=np.float32)
    bqkv = np.asarray(b_qkv, dtype=np.float32)
    wproj = np.asarray(W_proj, dtype=np.float32)
    bproj = np.ascontiguousarray(np.asarray(b_proj, dtype=np.float32))
    xts = [np.ascontiguousarray(X[b].T) for b in range(B)]
    in_maps = []
    for i in range(NCORES):
        b, hh = divmod(i, 2)
        xt = xts[b] if hh == 0 else np.ascontiguousarray(
            np.roll(xts[b], -SH, axis=1))
        o = 512 * hh
        wq = wqkv[:, o:o + 512]
        wk = wqkv[:, D + o:D + o + 512]
        wv = wqkv[:, 2 * D + o:2 * D + o + 512]
        w_core = np.ascontiguousarray(np.concatenate([wq, wk, wv], axis=1))
        bqk = np.concatenate([bqkv[o:o + 512], bqkv[D + o:D + o + 512]])
        bqkt = np.ascontiguousarray(bqk.reshape(8, 128).T)
        bv = np.ascontiguousarray(bqkv[2 * D + o:2 * D + o + 512])
        wp_core = np.ascontiguousarray(np.concatenate(
            [wproj[o:o + 512], wproj[512 * (1 - hh):512 * (1 - hh) + 512]],
            axis=0))
        in_maps.append({
            "XT": xt, "Wqkv": w_core, "bqkT": bqkt, "bv": bv,
            "Wproj": wp_core, "bproj": bproj,
        })
    return in_maps


_NC_CACHE = None


def get_nc():
    global _NC_CACHE
    if _NC_CACHE is None:
        _NC_CACHE = build_nc()
    return _NC_CACHE


def kernel(X, W_qkv, b_qkv, W_proj, b_proj):
    nc = get_nc()
    in_maps = make_in_maps(X, W_qkv, b_qkv, W_proj, b_proj)
    res = run_bass_kernel_spmd(nc, in_maps, core_ids=list(range(NCORES)))
    out = np.empty((B, S, D), np.float32)
    for i in range(NCORES):
        b, hh = divmod(i, 2)
        out[b, hh * SH:(hh + 1) * SH] = res.results[i]["out"]
    return out
